# revision 1
# baseline (speedup 1.0000x reference)
"""Trainium2 Bass kernel for nn_MultiHeadAttention_84473416778245.

Reference semantics (note two quirks):
  - softmax over the HEAD axis (axis=1), not the key axis -> purely
    elementwise per (q,k): attn[h] = exp(s[h]) / sum_h' exp(s[h'])
  - output reshape [B,H,S,hd] -> [B,S,H*hd] without transpose-back, which is
    a contiguous reinterpretation of the per-batch values buffer.

Sharding: data-parallel over batch B=8 -> one batch element per NeuronCore.

Device dataflow (per core, S=2048, H=8, hd=48, D=384):
  - host feeds x^T (augmented with a ones row for bias), packed transposed
    weights, all fp16
  - QKV projection on PE producing Q^T/K^T in [hd, S] layout (head pairs
    packed at partition bases 0 and 64) and V in [S, 48*H] layout
  - attention: per (q-block 512, k-tile 128):
      scores^T[k,q] per head via row-paired matmuls (K=48 at row groups 0/64)
      exp on ACT (scale fused), head-sum D via DVE adds, G = exp(-ln D) on
      ACT, attn = exp*G on DVE, PV via col-paired matmuls accumulating
      values^T[hd, q] in PSUM
  - out projection: out^T[e, s'] = sum_i W_o^T[48i:48i+48].T @ values^T[:, i::8]
    (the reshape quirk becomes a stride-8 column view), bias via per-partition
    ACT add; out^T DMA'd to HBM; host transposes after gather.
"""

import numpy as np
from contextlib import ExitStack

H, HD, D = 8, 48, 384
S_FULL = 2048
B = 8

_CACHE = {}


def _pack_host(W_qkv, b_qkv, W_o, b_o):
    f16 = np.float16
    wqkT = np.zeros((385, 1024), np.float32)
    for j in range(4):
        hA, hB = 2 * j, 2 * j + 1
        for t, row0 in ((2 * j, 48), (2 * j + 1, 0)):  # K tile, then Q tile
            for col0, h in ((0, hA), (64, hB)):
                rows = slice(144 * h + row0, 144 * h + row0 + 48)
                wqkT[:384, t * 128 + col0 : t * 128 + col0 + 48] = W_qkv[rows, :].T
                wqkT[384, t * 128 + col0 : t * 128 + col0 + 48] = b_qkv[rows]
    wvT = np.zeros((385, 384), np.float32)
    for h in range(H):
        rows = slice(144 * h + 96, 144 * h + 144)
        wvT[:384, 48 * h : 48 * h + 48] = W_qkv[rows, :].T
        wvT[384, 48 * h : 48 * h + 48] = b_qkv[rows]
    woT = np.zeros((128, 8 * 384), np.float32)
    WoT = np.ascontiguousarray(W_o.T)
    for i in range(8):
        woT[0:48, i * 384 : (i + 1) * 384] = WoT[48 * i : 48 * i + 48, :]
        woT[64:112, i * 384 : (i + 1) * 384] = WoT[48 * i : 48 * i + 48, :]
    bo = np.ascontiguousarray(b_o.astype(np.float32).reshape(3, 128, 1))
    return wqkT.astype(f16), wvT.astype(f16), woT.astype(f16), bo


def build_program(S=S_FULL, use_ln_recip=True, repeats=1, ablate=(), d_on_pe=False, exp2048=False, bigbufs=3):
    """Build the (single-core SPMD) Bass program. Returns compiled nc.

    repeats>1 re-runs the whole compute body serially (same tiles/tags), for
    slope-based HW timing: wall(R) ~ overhead + R * t_kernel."""
    import concourse.bass as bass  # noqa: F401
    import concourse.tile as tile
    from concourse import bacc, mybir

    f16 = mybir.dt.float16
    f32 = mybir.dt.float32
    AF = mybir.ActivationFunctionType

    QB = min(512, S)          # q block
    n_qb = S // QB
    n_kt = S // 128           # k tiles
    n_st = S // 128           # s tiles for V
    SC = min(512, S)          # s chunk for qk^T projection
    n_sc = S // SC
    T = S // 8                # out column block per head
    scale = float(1.0 / np.sqrt(48.0))

    # Force Exp and Ln to resolve to the combined 'natural_log_exp_and_others'
    # ACT table set: the greedy per-function set choice would otherwise
    # alternate exp_and_others <-> natural_log every attention tile (~2.7us
    # per table load). get_activation_tables is functools.cache'd and returns
    # the live dict, so mutate it in place; indices (act_func_set_id) are
    # positional and unchanged.
    from concourse import hw_specs

    _tables = hw_specs.get_activation_tables("gen3")
    for _name, _funcs in _tables.items():
        if _name != "natural_log_exp_and_others":
            _funcs.discard(mybir.ActivationFunctionType.Exp)
            _funcs.discard(mybir.ActivationFunctionType.Ln)

    nc = bacc.Bacc("TRN2", target_bir_lowering=False, debug=False)

    xT_d = nc.dram_tensor("xT", [385, S], f16, kind="ExternalInput").ap()
    wqk_d = nc.dram_tensor("wqkT", [385, 1024], f16, kind="ExternalInput").ap()
    wv_d = nc.dram_tensor("wvT", [385, 384], f16, kind="ExternalInput").ap()
    wo_d = nc.dram_tensor("woT", [128, 3072], f16, kind="ExternalInput").ap()
    bo_d = nc.dram_tensor("bo", [3, 128, 1], f32, kind="ExternalInput").ap()
    out_d = nc.dram_tensor("outT", [384, S], f32, kind="ExternalOutput").ap()

    with tile.TileContext(nc) as tc, ExitStack() as ctx:
        const = ctx.enter_context(tc.tile_pool(name="const", bufs=1))
        persist = ctx.enter_context(tc.tile_pool(name="persist", bufs=1))
        big2 = ctx.enter_context(tc.tile_pool(name="big2", bufs=bigbufs))
        small = ctx.enter_context(tc.tile_pool(name="small", bufs=4))
        outp = ctx.enter_context(tc.tile_pool(name="outp", bufs=2))

        # ---- load inputs -------------------------------------------------
        xT = [const.tile([128, S], f16, tag=f"xT{c}", name=f"xT{c}") for c in range(3)]
        xT1 = const.tile([1, S], f16, tag="xT3", name="xT3")
        for c in range(3):
            nc.sync.dma_start(xT[c][:], xT_d[128 * c : 128 * (c + 1), :])
        nc.sync.dma_start(xT1[:], xT_d[384:385, :])
        xch = xT + [xT1]

        wqk = [const.tile([128, 1024], f16, tag=f"wqk{c}", name=f"wqk{c}") for c in range(3)]
        wqk1 = const.tile([1, 1024], f16, tag="wqk3", name="wqk3")
        for c in range(3):
            nc.sync.dma_start(wqk[c][:], wqk_d[128 * c : 128 * (c + 1), :])
        nc.sync.dma_start(wqk1[:], wqk_d[384:385, :])
        wqkch = wqk + [wqk1]

        wv = [const.tile([128, 384], f16, tag=f"wv{c}", name=f"wv{c}") for c in range(3)]
        wv1 = const.tile([1, 384], f16, tag="wv3", name="wv3")
        for c in range(3):
            nc.sync.dma_start(wv[c][:], wv_d[128 * c : 128 * (c + 1), :])
        nc.sync.dma_start(wv1[:], wv_d[384:385, :])
        wvch = wv + [wv1]

        wo = const.tile([128, 3072], f16, tag="wo", name="wo")
        nc.sync.dma_start(wo[:], wo_d[:, :])
        bo = [const.tile([128, 1], f32, tag=f"bo{e}", name=f"bo{e}") for e in range(3)]
        for e in range(3):
            nc.sync.dma_start(bo[e][:], bo_d[e])

        # ---- compute body (optionally repeated for slope timing) ---------
        for _rep in range(repeats):
            build_body(nc, tc, mybir, AF, persist, big2, small, outp,
                       xch, wqkch, wvch, wo, bo, out_d,
                       S, QB, n_qb, n_kt, n_st, SC, n_sc, T, scale,
                       use_ln_recip, f16, f32, ablate, d_on_pe, exp2048)

    nc.compile()
    return nc


def build_body(nc, tc, mybir, AF, persist, big2, small, outp,
               xch, wqkch, wvch, wo, bo, out_d,
               S, QB, n_qb, n_kt, n_st, SC, n_sc, T, scale,
               use_ln_recip, f16, f32, ablate=(), d_on_pe=False, exp2048=False):
    if True:
        # ---- QKV projection ---------------------------------------------
        qkT = [persist.tile([128, S], f16, tag=f"qkT{t}", name=f"qkT{t}") for t in range(8)]
        V = [persist.tile([128, 384], f16, tag=f"V{st}", name=f"V{st}") for st in range(n_st)]

        with tc.tile_pool(name="qkvps", bufs=2, space="PSUM") as qp:  # per-tile bufs below
            for t in range(8):
                for sc in range(n_sc):
                    ps = qp.tile([128, SC], f32, tag="qk_ps", name="qk_ps", bufs=3)
                    for c in range(4):
                        nc.tensor.matmul(
                            ps[:],
                            wqkch[c][:, t * 128 : (t + 1) * 128],
                            xch[c][:, sc * SC : (sc + 1) * SC],
                            start=(c == 0),
                            stop=(c == 3),
                        )
                    nc.vector.tensor_copy(qkT[t][:, sc * SC : (sc + 1) * SC], ps[:])
            for st in range(n_st):
                ps = qp.tile([128, 384], f32, tag="v_ps", name="v_ps")
                for c in range(4):
                    nc.tensor.matmul(
                        ps[:],
                        xch[c][:, st * 128 : (st + 1) * 128],
                        wvch[c][:],
                        start=(c == 0),
                        stop=(c == 3),
                    )
                nc.scalar.copy(V[st][:], ps[:])

        # ---- attention ---------------------------------------------------
        values = [persist.tile([128, S], f16, tag=f"values{j}", name=f"values{j}") for j in range(4)]

        if d_on_pe:
            from concourse.masks import make_identity

            ident = persist.tile([128, 128], f16, tag="ident", name="ident")
            make_identity(nc, ident[:])

        with (
            tc.tile_pool(name="scps", bufs=(1 if (d_on_pe or exp2048) else 2), space="PSUM") as scp,
            tc.tile_pool(name="pvps", bufs=1, space="PSUM") as pvp,
            tc.tile_pool(name="dps", bufs=2, space="PSUM") as dpp,
        ):
            if "attn" in ablate:
                nc.sync.dma_start(out_d[0:128, 0:192], V[0][:].bitcast(f32))
                return
            for qb in range(n_qb):
                qs = slice(qb * QB, (qb + 1) * QB)
                vps = [pvp.tile([128, QB], f32, tag=f"pv{j}", name=f"pv{j}") for j in range(4)]
                for kt in range(n_kt):
                    ks = slice(kt * 128, (kt + 1) * 128)
                    exp_sb = big2.tile([128, 8 * QB], f16, tag="exp", name="exp_sb")
                    if exp2048:
                        for half in range(2):
                            sps = scp.tile(
                                [128, 2048], f32, tag="sc_ps", name="sc_ps"
                            )
                            for jj in range(2):
                                j = 2 * half + jj
                                nc.tensor.matmul(
                                    sps[:, 1024 * jj : 1024 * jj + QB],
                                    qkT[2 * j][0:48, ks],
                                    qkT[2 * j + 1][0:48, qs],
                                    tile_position=(0, 0),
                                )
                                nc.tensor.matmul(
                                    sps[:, 1024 * jj + 512 : 1024 * jj + 512 + QB],
                                    qkT[2 * j][64:112, ks],
                                    qkT[2 * j + 1][64:112, qs],
                                    tile_position=(64, 0),
                                )
                            sps_v = sps[:, :].rearrange(
                                "p (b q) -> p b q", b=4
                            )[:, :, 0:QB]
                            exp_v = exp_sb[
                                :, 4 * half * QB : 4 * (half + 1) * QB
                            ].rearrange("p (b q) -> p b q", b=4)
                            nc.scalar.activation(exp_v, sps_v, AF.Exp, scale=scale)
                    else:
                        for j in range(4):
                            # each half sits in its own PSUM bank (512 f32 cols)
                            sps = scp.tile([128, 1024], f32, tag="sc_ps", name="sc_ps")
                            nc.tensor.matmul(
                                sps[:, 0:QB],
                                qkT[2 * j][0:48, ks],
                                qkT[2 * j + 1][0:48, qs],
                                tile_position=(0, 0),
                            )
                            nc.tensor.matmul(
                                sps[:, 512 : 512 + QB],
                                qkT[2 * j][64:112, ks],
                                qkT[2 * j + 1][64:112, qs],
                                tile_position=(64, 0),
                            )
                            sps_v = sps[:, :].rearrange("p (b q) -> p b q", b=2)[
                                :, :, 0:QB
                            ]
                            exp_v = exp_sb[
                                :, 2 * j * QB : (2 * j + 2) * QB
                            ].rearrange("p (b q) -> p b q", b=2)
                            nc.scalar.activation(exp_v, sps_v, AF.Exp, scale=scale)
                    if "norm" in ablate:
                        attn = exp_sb
                    else:
                        attn = None
                    if attn is None and d_on_pe:
                        D_ps = dpp.tile([128, QB], f32, tag="D_ps", name="D_ps")
                        for h in range(8):
                            nc.tensor.matmul(
                                D_ps[:],
                                ident[:],
                                exp_sb[:, h * QB : (h + 1) * QB],
                                start=(h == 0),
                                stop=(h == 7),
                            )
                        Gt = small.tile([128, QB], f16, tag="G", name="Gt")
                        lnD = small.tile([128, QB], f16, tag="lnD", name="lnD")
                        nc.scalar.activation(lnD[:], D_ps[:], AF.Ln)
                        nc.scalar.activation(Gt[:], lnD[:], AF.Exp, scale=-1.0)
                    # D = sum over heads: 4-op tree; first two ops start as
                    # soon as exp waves 1 and 3 land (better pipelining)
                    elif attn is None:
                        D2 = small.tile([128, 4 * QB], f16, tag="D2", name="D2")
                        nc.vector.tensor_add(
                            D2[:, 0 : 2 * QB],
                            exp_sb[:, 0 : 2 * QB],
                            exp_sb[:, 2 * QB : 4 * QB],
                        )
                        nc.vector.tensor_add(
                            D2[:, 2 * QB : 4 * QB],
                            exp_sb[:, 4 * QB : 6 * QB],
                            exp_sb[:, 6 * QB : 8 * QB],
                        )
                        nc.vector.tensor_add(
                            D2[:, 0 : 2 * QB], D2[:, 0 : 2 * QB], D2[:, 2 * QB : 4 * QB]
                        )
                        Dt = small.tile([128, QB], f16, tag="D", name="Dt")
                        nc.vector.tensor_add(Dt[:], D2[:, 0:QB], D2[:, QB : 2 * QB])
                        Gt = small.tile([128, QB], f16, tag="G", name="Gt")
                        if use_ln_recip:
                            lnD = small.tile([128, QB], f16, tag="lnD", name="lnD")
                            nc.scalar.activation(lnD[:], Dt[:], AF.Ln)
                            nc.scalar.activation(Gt[:], lnD[:], AF.Exp, scale=-1.0)
                        else:
                            Df = small.tile([128, QB], f32, tag="Df", name="Df")
                            nc.vector.tensor_copy(Df[:], Dt[:])
                            Gf = small.tile([128, QB], f32, tag="Gf", name="Gf")
                            nc.vector.reciprocal_approx_fast(Gf[:], Df[:])
                            nc.vector.tensor_copy(Gt[:], Gf[:])
                    if attn is None:
                        attn = big2.tile([128, 8 * QB], f16, tag="attn", name="attn")
                        # fused muls: 2 ops of 4 heads each; G broadcast via a
                        # step-0 middle AP dim (innermost stays step-1 so the
                        # DVE 2x_1p mode is preserved)
                        g_b = Gt[:].rearrange("p (o q) -> p o q", o=1).broadcast_to([128, 4, QB])
                        for w in range(2):
                            nc.vector.tensor_mul(
                                attn[:, 4 * w * QB : 4 * (w + 1) * QB].rearrange(
                                    "p (h q) -> p h q", h=4
                                ),
                                exp_sb[:, 4 * w * QB : 4 * (w + 1) * QB].rearrange(
                                    "p (h q) -> p h q", h=4
                                ),
                                g_b,
                            )
                    if "pv" in ablate:
                        if kt == n_kt - 1:
                            nc.sync.dma_start(
                                out_d[0:128, :],
                                attn[:, 0 : 8 * QB].bitcast(f32)[:, 0 : S],
                            )
                        continue
                    for j in range(4):
                        nc.tensor.matmul(
                            vps[j][0:48, :],
                            V[kt][:, 96 * j : 96 * j + 48],
                            attn[:, 2 * j * QB : (2 * j + 1) * QB],
                            start=(kt == 0),
                            stop=(kt == n_kt - 1),
                            tile_position=(0, 0),
                        )
                        nc.tensor.matmul(
                            vps[j][64:112, :],
                            V[kt][:, 96 * j + 48 : 96 * j + 96],
                            attn[:, (2 * j + 1) * QB : (2 * j + 2) * QB],
                            start=(kt == 0),
                            stop=(kt == n_kt - 1),
                            tile_position=(0, 64),
                            # disjoint partition range (64:112) of the same
                            # bank as the (0,0) group; group check is
                            # bank-granular and would false-positive
                            skip_group_check=True,
                        )
                if "pv" in ablate:
                    continue
                for j in range(4):
                    nc.vector.tensor_copy(values[j][0:48, qs], vps[j][0:48, :])
                    nc.vector.tensor_copy(
                        values[j][64:112, qs], vps[j][64:112, :]
                    )

        if "pv" in ablate:
            return
        # ---- output projection -------------------------------------------
        # Each head owns a full 512-f32-col PSUM region (start=True lazily
        # zeroes the whole region), 4 heads per [128, 2048] tile, two halves
        # per e-tile. Even heads read values partitions 0:48 (row group 0),
        # odd heads partitions 64:112 (row group 64, duplicated W_o^T rows).
        with tc.tile_pool(name="outps", bufs=2, space="PSUM") as op:
            for et in range(3):
                osb = outp.tile([128, S], f32, tag="osb", name="osb")
                for half in range(2):
                    ops_ = op.tile([128, 2048], f32, tag="out_ps", name="out_ps")
                    for hh in range(4):
                        h = 4 * half + hh
                        base = 0 if h % 2 == 0 else 64
                        rhs = values[h // 2][base : base + 48, :].rearrange(
                            "p (t i) -> p i t", i=8
                        )
                        for i in range(8):
                            nc.tensor.matmul(
                                ops_[:, 512 * hh : 512 * hh + T],
                                wo[
                                    base : base + 48,
                                    i * 384 + et * 128 : i * 384 + et * 128 + 128,
                                ],
                                rhs[:, i, :],
                                start=(i == 0),
                                stop=(i == 7),
                                tile_position=(base, 0),
                            )
                    ops_v = ops_[:, :].rearrange("p (hh q) -> p hh q", hh=4)[
                        :, :, 0:T
                    ]
                    osb_v = osb[
                        :, 4 * half * T : 4 * (half + 1) * T
                    ].rearrange("p (hh q) -> p hh q", hh=4)
                    nc.scalar.activation(
                        osb_v, ops_v, AF.Identity, bias=bo[et][:]
                    )
                nc.sync.dma_start(out_d[et * 128 : (et + 1) * 128, :], osb[:])


def _get_program(S=S_FULL):
    key = ("nc", S)
    if key not in _CACHE:
        _CACHE[key] = build_program(S)
    return _CACHE[key]


def kernel(x, W_qkv, b_qkv, W_o, b_o):
    from concourse import bass_utils

    x = np.asarray(x, dtype=np.float32)
    W_qkv = np.asarray(W_qkv, dtype=np.float32)
    b_qkv = np.asarray(b_qkv, dtype=np.float32)
    W_o = np.asarray(W_o, dtype=np.float32)
    b_o = np.asarray(b_o, dtype=np.float32)
    Bx, S, _ = x.shape

    wqkT, wvT, woT, bo = _pack_host(W_qkv, b_qkv, W_o, b_o)
    in_maps = []
    for b in range(Bx):
        xT = np.ones((385, S), np.float32)
        xT[:384] = x[b].T
        in_maps.append(
            {
                "xT": xT.astype(np.float16),
                "wqkT": wqkT,
                "wvT": wvT,
                "woT": woT,
                "bo": bo,
            }
        )

    nc = _get_program(S)
    res = bass_utils.run_bass_kernel_spmd(nc, in_maps, core_ids=list(range(Bx)))
    out = np.stack([np.ascontiguousarray(r["outT"].T) for r in res.results])
    return out.astype(np.float32)



# revision 23
# speedup vs baseline: 22.8854x; 22.8854x over previous
"""Trainium2 Bass kernel for nn_MultiHeadAttention_84473416778245.

Reference semantics (note two quirks):
  - softmax over the HEAD axis (axis=1), not the key axis -> purely
    elementwise per (q,k): attn[h] = exp(s[h]) / sum_h' exp(s[h'])
  - output reshape [B,H,S,hd] -> [B,S,H*hd] without transpose-back, which is
    a contiguous reinterpretation of the per-batch values buffer.

Sharding: data-parallel over batch B=8 -> one batch element per NeuronCore.

Device dataflow (per core, S=2048, H=8, hd=48, D=384):
  - host feeds x^T (augmented with a ones row for bias), packed transposed
    weights, all fp16
  - QKV projection on PE producing Q^T/K^T in [hd, S] layout (head pairs
    packed at partition bases 0 and 64) and V in [S, 48*H] layout
  - attention: per (q-block 512, k-tile 128):
      scores^T[k,q] per head via row-paired matmuls (K=48 at row groups 0/64)
      exp on ACT (scale fused), head-sum D via DVE adds, G = exp(-ln D) on
      ACT, attn = exp*G on DVE, PV via col-paired matmuls accumulating
      values^T[hd, q] in PSUM
  - out projection: out^T[e, s'] = sum_i W_o^T[48i:48i+48].T @ values^T[:, i::8]
    (the reshape quirk becomes a stride-8 column view), bias via per-partition
    ACT add; out^T DMA'd to HBM; host transposes after gather.
"""

import numpy as np
from contextlib import ExitStack

H, HD, D = 8, 48, 384
S_FULL = 2048
B = 8

_CACHE = {}


def _pack_host(W_qkv, b_qkv, W_o, b_o):
    f16 = np.float16
    wqkT = np.zeros((385, 1024), np.float32)
    for j in range(4):
        hA, hB = 2 * j, 2 * j + 1
        for t, row0 in ((2 * j, 48), (2 * j + 1, 0)):  # K tile, then Q tile
            for col0, h in ((0, hA), (64, hB)):
                rows = slice(144 * h + row0, 144 * h + row0 + 48)
                wqkT[:384, t * 128 + col0 : t * 128 + col0 + 48] = W_qkv[rows, :].T
                wqkT[384, t * 128 + col0 : t * 128 + col0 + 48] = b_qkv[rows]
    wvT = np.zeros((385, 384), np.float32)
    for h in range(H):
        rows = slice(144 * h + 96, 144 * h + 144)
        wvT[:384, 48 * h : 48 * h + 48] = W_qkv[rows, :].T
        wvT[384, 48 * h : 48 * h + 48] = b_qkv[rows]
    woT = np.zeros((128, 8 * 384), np.float32)
    WoT = np.ascontiguousarray(W_o.T)
    for i in range(8):
        woT[0:48, i * 384 : (i + 1) * 384] = WoT[48 * i : 48 * i + 48, :]
        woT[64:112, i * 384 : (i + 1) * 384] = WoT[48 * i : 48 * i + 48, :]
    bo = np.ascontiguousarray(b_o.astype(np.float32).reshape(3, 128, 1))
    qkb = np.zeros((128, 8), np.float32)
    for j in range(4):
        hA, hB = 2 * j, 2 * j + 1
        for t, row0 in ((2 * j, 48), (2 * j + 1, 0)):
            for col0, h in ((0, hA), (64, hB)):
                qkb[col0 : col0 + 48, t] = b_qkv[144 * h + row0 : 144 * h + row0 + 48]
    return wqkT.astype(f16), wvT.astype(f16), woT.astype(f16), bo, qkb


def build_program(S=S_FULL, use_ln_recip=True, repeats=1, ablate=(), d_on_pe=False, exp2048=False, bigbufs=6, v2=True, gp_adds=0, interleave=True, filler=0, g_dve=False):
    """Build the (single-core SPMD) Bass program. Returns compiled nc.

    repeats>1 re-runs the whole compute body serially (same tiles/tags), for
    slope-based HW timing: wall(R) ~ overhead + R * t_kernel."""
    import concourse.bass as bass  # noqa: F401
    import concourse.tile as tile
    from concourse import bacc, mybir

    f16 = mybir.dt.float16
    f32 = mybir.dt.float32
    AF = mybir.ActivationFunctionType

    QB = min(512, S)          # q block
    n_qb = S // QB
    n_kt = S // 128           # k tiles
    n_st = S // 128           # s tiles for V
    SC = min(512, S)          # s chunk for qk^T projection
    n_sc = S // SC
    T = S // 8                # out column block per head
    scale = float(1.0 / np.sqrt(48.0))

    # Force Exp and Ln to resolve to the combined 'natural_log_exp_and_others'
    # ACT table set: the greedy per-function set choice would otherwise
    # alternate exp_and_others <-> natural_log every attention tile (~2.7us
    # per table load). get_activation_tables is functools.cache'd and returns
    # the live dict, so mutate it in place; indices (act_func_set_id) are
    # positional and unchanged.
    from concourse import hw_specs

    _tables = hw_specs.get_activation_tables("gen3")
    for _name, _funcs in _tables.items():
        if _name != "natural_log_exp_and_others":
            _funcs.discard(mybir.ActivationFunctionType.Exp)
            _funcs.discard(mybir.ActivationFunctionType.Ln)

    nc = bacc.Bacc("TRN2", target_bir_lowering=False, debug=False)

    xT_d = nc.dram_tensor("xT", [385, S], f16, kind="ExternalInput").ap()
    wqk_d = nc.dram_tensor("wqkT", [385, 1024], f16, kind="ExternalInput").ap()
    wv_d = nc.dram_tensor("wvT", [385, 384], f16, kind="ExternalInput").ap()
    wo_d = nc.dram_tensor("woT", [128, 3072], f16, kind="ExternalInput").ap()
    qkb_d = nc.dram_tensor("qkb", [128, 8], f32, kind="ExternalInput").ap()
    bo_d = nc.dram_tensor("bo", [3, 128, 1], f32, kind="ExternalInput").ap()
    out_d = nc.dram_tensor("outT", [384, S], f32, kind="ExternalOutput").ap()

    with tile.TileContext(nc) as tc, ExitStack() as ctx:
        const = ctx.enter_context(tc.tile_pool(name="const", bufs=1))
        persist = ctx.enter_context(tc.tile_pool(name="persist", bufs=1))
        big2 = ctx.enter_context(tc.tile_pool(name="big2", bufs=bigbufs))
        small = ctx.enter_context(tc.tile_pool(name="small", bufs=4))
        outp = ctx.enter_context(tc.tile_pool(name="outp", bufs=2))

        # ---- load inputs -------------------------------------------------
        # weights first, then x chunk 0, so the first QKV matmul can start
        # after ~1.2MB of DMA instead of the full input set; x is split into
        # per-chunk tiles so the Tile dep tracker releases chunk 0 early.
        n_ch = S // SC
        wqk = [const.tile([128, 1024], f16, tag=f"wqk{c}", name=f"wqk{c}") for c in range(3)]
        for c in range(3):
            nc.sync.dma_start(wqk[c][:], wqk_d[128 * c : 128 * (c + 1), :])
        wqkch = wqk

        xT = [
            [const.tile([128, SC], f16, tag=f"xT{c}_{k}", name=f"xT{c}_{k}") for k in range(n_ch)]
            for c in range(3)
        ]
        for k in range(n_ch):
            for c in range(3):
                nc.sync.dma_start(
                    xT[c][k][:], xT_d[128 * c : 128 * (c + 1), k * SC : (k + 1) * SC]
                )
        xch = xT

        wv = [const.tile([128, 384], f16, tag=f"wv{c}", name=f"wv{c}") for c in range(3)]
        wv1 = const.tile([1, 384], f16, tag="wv3", name="wv3")
        for c in range(3):
            nc.sync.dma_start(wv[c][:], wv_d[128 * c : 128 * (c + 1), :])
        nc.sync.dma_start(wv1[:], wv_d[384:385, :])
        wvch = wv + [wv1]

        qkb = const.tile([128, 8], f32, tag="qkb", name="qkb")
        nc.sync.dma_start(qkb[:], qkb_d[:, :])
        xones = const.tile([1, S], f16, tag="xones", name="xones")
        nc.vector.memset(xones[:], 1.0)
        wo = const.tile([128, 3072], f16, tag="wo", name="wo")
        nc.sync.dma_start(wo[:], wo_d[:, :])
        bo = [const.tile([128, 1], f32, tag=f"bo{e}", name=f"bo{e}") for e in range(3)]
        for e in range(3):
            nc.sync.dma_start(bo[e][:], bo_d[e])

        # ---- compute body (optionally repeated for slope timing) ---------
        for _rep in range(repeats):
            if v2:
                build_body_v2(nc, tc, mybir, AF, persist, big2, small, outp,
                              xch, wqkch, wvch, wo, bo, out_d,
                              S, QB, n_qb, n_kt, n_st, SC, n_sc, T, scale,
                              f16, f32, gp_adds, interleave, filler, qkb, g_dve, xones)
            else:
                build_body(nc, tc, mybir, AF, persist, big2, small, outp,
                           xch, wqkch, wvch, wo, bo, out_d,
                           S, QB, n_qb, n_kt, n_st, SC, n_sc, T, scale,
                           use_ln_recip, f16, f32, ablate, d_on_pe, exp2048)

    nc.compile()
    return nc


def build_body(nc, tc, mybir, AF, persist, big2, small, outp,
               xch, wqkch, wvch, wo, bo, out_d,
               S, QB, n_qb, n_kt, n_st, SC, n_sc, T, scale,
               use_ln_recip, f16, f32, ablate=(), d_on_pe=False, exp2048=False):
    if True:
        # ---- QKV projection ---------------------------------------------
        qkT = [persist.tile([128, S], f16, tag=f"qkT{t}", name=f"qkT{t}") for t in range(8)]
        V = [persist.tile([128, 384], f16, tag=f"V{st}", name=f"V{st}") for st in range(n_st)]

        with tc.tile_pool(name="qkvps", bufs=2, space="PSUM") as qp:  # per-tile bufs below
            for t in range(8):
                for sc in range(n_sc):
                    ps = qp.tile([128, SC], f32, tag="qk_ps", name="qk_ps", bufs=3)
                    for c in range(4):
                        nc.tensor.matmul(
                            ps[:],
                            wqkch[c][:, t * 128 : (t + 1) * 128],
                            xch[c][:, sc * SC : (sc + 1) * SC],
                            start=(c == 0),
                            stop=(c == 3),
                        )
                    nc.vector.tensor_copy(qkT[t][:, sc * SC : (sc + 1) * SC], ps[:])
            for st in range(n_st):
                ps = qp.tile([128, 384], f32, tag="v_ps", name="v_ps")
                for c in range(4):
                    nc.tensor.matmul(
                        ps[:],
                        xch[c][:, st * 128 : (st + 1) * 128],
                        wvch[c][:],
                        start=(c == 0),
                        stop=(c == 3),
                    )
                nc.scalar.copy(V[st][:], ps[:])

        # ---- attention ---------------------------------------------------
        values = [persist.tile([128, S], f16, tag=f"values{j}", name=f"values{j}") for j in range(4)]

        if d_on_pe:
            from concourse.masks import make_identity

            ident = persist.tile([128, 128], f16, tag="ident", name="ident")
            make_identity(nc, ident[:])

        with (
            tc.tile_pool(name="scps", bufs=(1 if (d_on_pe or exp2048) else 2), space="PSUM") as scp,
            tc.tile_pool(name="pvps", bufs=1, space="PSUM") as pvp,
            tc.tile_pool(name="dps", bufs=2, space="PSUM") as dpp,
        ):
            if "attn" in ablate:
                nc.sync.dma_start(out_d[0:128, 0:192], V[0][:].bitcast(f32))
                return
            for qb in range(n_qb):
                qs = slice(qb * QB, (qb + 1) * QB)
                vps = [pvp.tile([128, QB], f32, tag=f"pv{j}", name=f"pv{j}") for j in range(4)]
                for kt in range(n_kt):
                    ks = slice(kt * 128, (kt + 1) * 128)
                    exp_sb = big2.tile([128, 8 * QB], f16, tag="exp", name="exp_sb")
                    if exp2048:
                        for half in range(2):
                            sps = scp.tile(
                                [128, 2048], f32, tag="sc_ps", name="sc_ps"
                            )
                            for jj in range(2):
                                j = 2 * half + jj
                                nc.tensor.matmul(
                                    sps[:, 1024 * jj : 1024 * jj + QB],
                                    qkT[2 * j][0:48, ks],
                                    qkT[2 * j + 1][0:48, qs],
                                    tile_position=(0, 0),
                                )
                                nc.tensor.matmul(
                                    sps[:, 1024 * jj + 512 : 1024 * jj + 512 + QB],
                                    qkT[2 * j][64:112, ks],
                                    qkT[2 * j + 1][64:112, qs],
                                    tile_position=(64, 0),
                                )
                            sps_v = sps[:, :].rearrange(
                                "p (b q) -> p b q", b=4
                            )[:, :, 0:QB]
                            exp_v = exp_sb[
                                :, 4 * half * QB : 4 * (half + 1) * QB
                            ].rearrange("p (b q) -> p b q", b=4)
                            nc.scalar.activation(exp_v, sps_v, AF.Exp, scale=scale)
                    else:
                        for j in range(4):
                            # each half sits in its own PSUM bank (512 f32 cols)
                            sps = scp.tile([128, 1024], f32, tag="sc_ps", name="sc_ps")
                            nc.tensor.matmul(
                                sps[:, 0:QB],
                                qkT[2 * j][0:48, ks],
                                qkT[2 * j + 1][0:48, qs],
                                tile_position=(0, 0),
                            )
                            nc.tensor.matmul(
                                sps[:, 512 : 512 + QB],
                                qkT[2 * j][64:112, ks],
                                qkT[2 * j + 1][64:112, qs],
                                tile_position=(64, 0),
                            )
                            sps_v = sps[:, :].rearrange("p (b q) -> p b q", b=2)[
                                :, :, 0:QB
                            ]
                            exp_v = exp_sb[
                                :, 2 * j * QB : (2 * j + 2) * QB
                            ].rearrange("p (b q) -> p b q", b=2)
                            nc.scalar.activation(exp_v, sps_v, AF.Exp, scale=scale)
                    if "norm" in ablate:
                        attn = exp_sb
                    else:
                        attn = None
                    if attn is None and d_on_pe:
                        D_ps = dpp.tile([128, QB], f32, tag="D_ps", name="D_ps")
                        for h in range(8):
                            nc.tensor.matmul(
                                D_ps[:],
                                ident[:],
                                exp_sb[:, h * QB : (h + 1) * QB],
                                start=(h == 0),
                                stop=(h == 7),
                            )
                        Gt = small.tile([128, QB], f16, tag="G", name="Gt")
                        lnD = small.tile([128, QB], f16, tag="lnD", name="lnD")
                        nc.scalar.activation(lnD[:], D_ps[:], AF.Ln)
                        nc.scalar.activation(Gt[:], lnD[:], AF.Exp, scale=-1.0)
                    # D = sum over heads: 4-op tree; first two ops start as
                    # soon as exp waves 1 and 3 land (better pipelining)
                    elif attn is None:
                        D2 = small.tile([128, 4 * QB], f16, tag="D2", name="D2")
                        nc.vector.tensor_add(
                            D2[:, 0 : 2 * QB],
                            exp_sb[:, 0 : 2 * QB],
                            exp_sb[:, 2 * QB : 4 * QB],
                        )
                        nc.vector.tensor_add(
                            D2[:, 2 * QB : 4 * QB],
                            exp_sb[:, 4 * QB : 6 * QB],
                            exp_sb[:, 6 * QB : 8 * QB],
                        )
                        nc.vector.tensor_add(
                            D2[:, 0 : 2 * QB], D2[:, 0 : 2 * QB], D2[:, 2 * QB : 4 * QB]
                        )
                        Dt = small.tile([128, QB], f16, tag="D", name="Dt")
                        nc.vector.tensor_add(Dt[:], D2[:, 0:QB], D2[:, QB : 2 * QB])
                        Gt = small.tile([128, QB], f16, tag="G", name="Gt")
                        if use_ln_recip:
                            lnD = small.tile([128, QB], f16, tag="lnD", name="lnD")
                            nc.scalar.activation(lnD[:], Dt[:], AF.Ln)
                            nc.scalar.activation(Gt[:], lnD[:], AF.Exp, scale=-1.0)
                        else:
                            Df = small.tile([128, QB], f32, tag="Df", name="Df")
                            nc.vector.tensor_copy(Df[:], Dt[:])
                            Gf = small.tile([128, QB], f32, tag="Gf", name="Gf")
                            nc.vector.reciprocal_approx_fast(Gf[:], Df[:])
                            nc.vector.tensor_copy(Gt[:], Gf[:])
                    if attn is None:
                        attn = big2.tile([128, 8 * QB], f16, tag="attn", name="attn")
                        # fused muls: 2 ops of 4 heads each; G broadcast via a
                        # step-0 middle AP dim (innermost stays step-1 so the
                        # DVE 2x_1p mode is preserved)
                        g_b = Gt[:].rearrange("p (o q) -> p o q", o=1).broadcast_to([128, 4, QB])
                        for w in range(2):
                            nc.vector.tensor_mul(
                                attn[:, 4 * w * QB : 4 * (w + 1) * QB].rearrange(
                                    "p (h q) -> p h q", h=4
                                ),
                                exp_sb[:, 4 * w * QB : 4 * (w + 1) * QB].rearrange(
                                    "p (h q) -> p h q", h=4
                                ),
                                g_b,
                            )
                    if "pv" in ablate:
                        if kt == n_kt - 1:
                            nc.sync.dma_start(
                                out_d[0:128, :],
                                attn[:, 0 : 8 * QB].bitcast(f32)[:, 0 : S],
                            )
                        continue
                    for j in range(4):
                        nc.tensor.matmul(
                            vps[j][0:48, :],
                            V[kt][:, 96 * j : 96 * j + 48],
                            attn[:, 2 * j * QB : (2 * j + 1) * QB],
                            start=(kt == 0),
                            stop=(kt == n_kt - 1),
                            tile_position=(0, 0),
                        )
                        nc.tensor.matmul(
                            vps[j][64:112, :],
                            V[kt][:, 96 * j + 48 : 96 * j + 96],
                            attn[:, (2 * j + 1) * QB : (2 * j + 2) * QB],
                            start=(kt == 0),
                            stop=(kt == n_kt - 1),
                            tile_position=(0, 64),
                            # disjoint partition range (64:112) of the same
                            # bank as the (0,0) group; group check is
                            # bank-granular and would false-positive
                            skip_group_check=True,
                        )
                if "pv" in ablate:
                    continue
                for j in range(4):
                    nc.vector.tensor_copy(values[j][0:48, qs], vps[j][0:48, :])
                    nc.vector.tensor_copy(
                        values[j][64:112, qs], vps[j][64:112, :]
                    )

        if "pv" in ablate:
            return
        # ---- output projection -------------------------------------------
        # Each head owns a full 512-f32-col PSUM region (start=True lazily
        # zeroes the whole region), 4 heads per [128, 2048] tile, two halves
        # per e-tile. Even heads read values partitions 0:48 (row group 0),
        # odd heads partitions 64:112 (row group 64, duplicated W_o^T rows).
        with tc.tile_pool(name="outps", bufs=2, space="PSUM") as op:
            for et in range(3):
                osb = outp.tile([128, S], f32, tag="osb", name="osb")
                for half in range(2):
                    ops_ = op.tile([128, 2048], f32, tag="out_ps", name="out_ps")
                    for hh in range(4):
                        h = 4 * half + hh
                        base = 0 if h % 2 == 0 else 64
                        rhs = values[h // 2][base : base + 48, :].rearrange(
                            "p (t i) -> p i t", i=8
                        )
                        for i in range(8):
                            nc.tensor.matmul(
                                ops_[:, 512 * hh : 512 * hh + T],
                                wo[
                                    base : base + 48,
                                    i * 384 + et * 128 : i * 384 + et * 128 + 128,
                                ],
                                rhs[:, i, :],
                                start=(i == 0),
                                stop=(i == 7),
                                tile_position=(base, 0),
                            )
                    ops_v = ops_[:, :].rearrange("p (hh q) -> p hh q", hh=4)[
                        :, :, 0:T
                    ]
                    osb_v = osb[
                        :, 4 * half * T : 4 * (half + 1) * T
                    ].rearrange("p (hh q) -> p hh q", hh=4)
                    nc.scalar.activation(
                        osb_v, ops_v, AF.Identity, bias=bo[et][:]
                    )
                nc.sync.dma_start(out_d[et * 128 : (et + 1) * 128, :], osb[:])


def build_body_v2(nc, tc, mybir, AF, persist, big2, small, outp,
                  xch, wqkch, wvch, wo, bo, out_d,
                  S, QB, n_qb, n_kt, n_st, SC, n_sc, T, scale,
                  f16, f32, gp_adds=1, interleave=True, filler=0, qkb=None, g_dve=False, xones=None):
    from contextlib import ExitStack as _ES
    _fes = _ES()

    def emit_filler(n):
        # dummy weight loads: pure PE-array activity (no PSUM write, own
        # SBUF read port) to keep the HAM activity monitor from
        # re-throttling the PE clock during ACT/DVE-paced stretches.
        for _ in range(n):
            nc.tensor.ldweights(wo[0:128, 0:512])
    """Rebalanced attention pipeline:
      - one D-tree add offloaded to GPSIMD (Pool) to unload DVE
      - values stored head-interleaved (i-major) so the out-projection
        streams a contiguous rhs instead of a stride-8 view
      - out-projection per-head rhs is then values[j][base:, i*T:(i+1)*T]
    """
    # ---- QKV projection + attention (interleaved) --------------------
    qkT = [persist.tile([128, S], f16, tag=f"qkT{t}", name=f"qkT{t}") for t in range(8)]
    V = [persist.tile([128, 384], f16, tag=f"V{st}", name=f"V{st}") for st in range(n_st)]
    values = [persist.tile([128, S], f16, tag=f"values{j}", name=f"values{j}") for j in range(4)]

    with tc.tile_pool(name="qkvps", bufs=2, space="PSUM") as qp:
        for sc in range(n_sc):
            for t in range(8):
                ps = qp.tile([128, SC], f32, tag="qk_ps", name="qk_ps", bufs=3)
                for c in range(3):
                    nc.tensor.matmul(
                        ps[:],
                        wqkch[c][:, t * 128 : (t + 1) * 128],
                        xch[c][sc][:],
                        start=(c == 0),
                        stop=(c == 2),
                    )
                # Q/K bias is per-partition in this layout: fold it into the
                # PSUM->SBUF copy instead of a 4th (1-row) matmul chunk.
                nc.vector.tensor_scalar_add(
                    qkT[t][:, sc * SC : (sc + 1) * SC], ps[:],
                    qkb[:, t : t + 1],
                )
            for st in range(sc * n_st // n_sc, (sc + 1) * n_st // n_sc):
                ps = qp.tile([128, 384], f32, tag="v_ps", name="v_ps")
                kk, off = st // (n_st // n_sc), (st % (n_st // n_sc)) * 128
                for c in range(4):
                    nc.tensor.matmul(
                        ps[:],
                        xones[:, st * 128 : (st + 1) * 128]
                        if c == 3
                        else xch[c][kk][:, off : off + 128],
                        wvch[c][:],
                        start=(c == 0),
                        stop=(c == 3),
                    )
                nc.scalar.copy(V[st][:], ps[:])

    with (
        tc.tile_pool(name="scps", bufs=2, space="PSUM") as scp,
        tc.tile_pool(name="pvps", bufs=1, space="PSUM") as pvp,
    ):
        for qb in range(n_qb):
            qs = slice(qb * QB, (qb + 1) * QB)
            vps = [pvp.tile([128, QB], f32, tag=f"pv{j}", name=f"pv{j}") for j in range(4)]

            def emit_pv(attn_t, kt):
                for j in range(4):
                    nc.tensor.matmul(
                        vps[j][0:48, :],
                        V[kt][:, 96 * j : 96 * j + 48],
                        attn_t[:, 2 * j * QB : (2 * j + 1) * QB],
                        start=(kt == 0),
                        stop=(kt == n_kt - 1),
                        tile_position=(0, 0),
                    )
                    nc.tensor.matmul(
                        vps[j][64:112, :],
                        V[kt][:, 96 * j + 48 : 96 * j + 96],
                        attn_t[:, (2 * j + 1) * QB : (2 * j + 2) * QB],
                        start=(kt == 0),
                        stop=(kt == n_kt - 1),
                        tile_position=(0, 64),
                        skip_group_check=True,
                    )
                    if filler:
                        emit_filler(filler)

            # Software pipeline, per iteration kt:
            #   PE:  scores(kt), then PV(kt-1) -- PV issued after the next
            #        tile's score matmuls so the PE's in-order queue isn't
            #        head-of-line blocked on kt-1's normalized weights
            #   ACT: exps(kt), then Ln/ExpG(kt-1)
            #   DVE: D-tree adds(kt), then in-place normalize muls(kt-1)
            # (Deeper deferral measures WORSE: longer PE idle stretches
            # trigger more HAM clock re-throttles.)
            pending = None
            for kt in range(n_kt):
                if True:
                    ks = slice(kt * 128, (kt + 1) * 128)
                    exp_sb = big2.tile([128, 8 * QB], f16, tag="exp", name="exp_sb")
                    for j in range(4):
                        # each half sits in its own PSUM bank (512 f32 cols)
                        sps = scp.tile([128, 1024], f32, tag="sc_ps", name="sc_ps")
                        nc.tensor.matmul(
                            sps[:, 0:QB],
                            qkT[2 * j][0:48, ks],
                            qkT[2 * j + 1][0:48, qs],
                            tile_position=(0, 0),
                        )
                        nc.tensor.matmul(
                            sps[:, 512 : 512 + QB],
                            qkT[2 * j][64:112, ks],
                            qkT[2 * j + 1][64:112, qs],
                            tile_position=(64, 0),
                        )
                        sps_v = sps[:, :].rearrange("p (b q) -> p b q", b=2)[
                            :, :, 0:QB
                        ]
                        exp_v = exp_sb[
                            :, 2 * j * QB : (2 * j + 2) * QB
                        ].rearrange("p (h q) -> p h q", h=2)
                        nc.scalar.activation(exp_v, sps_v, AF.Exp, scale=scale)
                # PV for the previous tile goes here, after this tile's
                # score matmuls, so the PE's in-order queue isn't blocked
                if pending is not None:
                    emit_pv(*pending)
                # D = sum over heads: 4-op tree
                D2 = small.tile([128, 4 * QB], f16, tag="D2", name="D2")
                nc.vector.tensor_add(
                    D2[:, 0 : 2 * QB],
                    exp_sb[:, 0 : 2 * QB],
                    exp_sb[:, 2 * QB : 4 * QB],
                )
                nc.vector.tensor_add(
                    D2[:, 2 * QB : 4 * QB],
                    exp_sb[:, 4 * QB : 6 * QB],
                    exp_sb[:, 6 * QB : 8 * QB],
                )
                nc.vector.tensor_add(
                    D2[:, 0 : 2 * QB], D2[:, 0 : 2 * QB], D2[:, 2 * QB : 4 * QB]
                )
                Dt = small.tile([128, QB], f16, tag="D", name="Dt")
                nc.vector.tensor_add(Dt[:], D2[:, 0:QB], D2[:, QB : 2 * QB])
                Gt = small.tile([128, QB], f16, tag="G", name="Gt")
                lnD = small.tile([128, QB], f16, tag="lnD", name="lnD")
                nc.scalar.activation(lnD[:], Dt[:], AF.Ln)
                nc.scalar.activation(Gt[:], lnD[:], AF.Exp, scale=-1.0)
                g_b = Gt[:].rearrange("p (o q) -> p o q", o=1).broadcast_to(
                    [128, 4, QB]
                )
                for w in range(2):
                    # in place: DVE is in-order, the D-tree adds above
                    # already read these columns
                    nc.vector.tensor_mul(
                        exp_sb[:, 4 * w * QB : 4 * (w + 1) * QB].rearrange(
                            "p (h q) -> p h q", h=4
                        ),
                        exp_sb[:, 4 * w * QB : 4 * (w + 1) * QB].rearrange(
                            "p (h q) -> p h q", h=4
                        ),
                        g_b,
                    )
                pending = (exp_sb, kt)
            emit_pv(*pending)
            # unload PV accumulators. With interleave=True, scatter the q
            # columns i-major (col i*T + t holds q=8t+i) so the out
            # projection's per-i rhs blocks are contiguous.
            U = QB // 8
            for j in range(4):
                for base in (0, 64):
                    src = vps[j][base : base + 48, :]
                    if interleave:
                        src_v = src.rearrange("p (u i) -> p i u", i=8)
                        dst_v = (
                            values[j][base : base + 48, :]
                            .rearrange("p (i t) -> p i t", i=8)[
                                :, :, qb * U : (qb + 1) * U
                            ]
                        )
                        nc.vector.tensor_copy(dst_v, src_v)
                    else:
                        nc.vector.tensor_copy(
                            values[j][base : base + 48, qs], src
                        )

    # ---- output projection -------------------------------------------
    with tc.tile_pool(name="outps", bufs=2, space="PSUM") as op:
        for et in range(3):
            osb = outp.tile([128, S], f32, tag="osb", name="osb")
            for half in range(2):
                ops_ = op.tile([128, 2048], f32, tag="out_ps", name="out_ps")
                for hh in range(4):
                    h = 4 * half + hh
                    base = 0 if h % 2 == 0 else 64
                    vt = values[h // 2]
                    for i in range(8):
                        if interleave:
                            rhs = vt[base : base + 48, i * T : (i + 1) * T]
                        else:
                            rhs = vt[base : base + 48, :].rearrange(
                                "p (t i) -> p i t", i=8
                            )[:, i, :]
                        nc.tensor.matmul(
                            ops_[:, 512 * hh : 512 * hh + T],
                            wo[
                                base : base + 48,
                                i * 384 + et * 128 : i * 384 + et * 128 + 128,
                            ],
                            rhs,
                            start=(i == 0),
                            stop=(i == 7),
                            tile_position=(base, 0),
                        )
                ops_v = ops_[:, :].rearrange("p (hh q) -> p hh q", hh=4)[
                    :, :, 0:T
                ]
                osb_v = osb[
                    :, 4 * half * T : 4 * (half + 1) * T
                ].rearrange("p (hh q) -> p hh q", hh=4)
                nc.scalar.activation(
                    osb_v, ops_v, AF.Identity, bias=bo[et][:]
                )
            nc.sync.dma_start(out_d[et * 128 : (et + 1) * 128, :], osb[:])


def _get_program(S=S_FULL):
    key = ("nc", S)
    if key not in _CACHE:
        _CACHE[key] = build_program(S)
    return _CACHE[key]


def kernel(x, W_qkv, b_qkv, W_o, b_o):
    from concourse import bass_utils

    x = np.asarray(x, dtype=np.float32)
    W_qkv = np.asarray(W_qkv, dtype=np.float32)
    b_qkv = np.asarray(b_qkv, dtype=np.float32)
    W_o = np.asarray(W_o, dtype=np.float32)
    b_o = np.asarray(b_o, dtype=np.float32)
    Bx, S, _ = x.shape

    wqkT, wvT, woT, bo, qkb = _pack_host(W_qkv, b_qkv, W_o, b_o)
    in_maps = []
    for b in range(Bx):
        xT = np.ones((385, S), np.float32)
        xT[:384] = x[b].T
        in_maps.append(
            {
                "xT": xT.astype(np.float16),
                "wqkT": wqkT,
                "wvT": wvT,
                "woT": woT,
                "bo": bo,
                "qkb": qkb,
            }
        )

    nc = _get_program(S)
    res = bass_utils.run_bass_kernel_spmd(nc, in_maps, core_ids=list(range(Bx)))
    out = np.stack([np.ascontiguousarray(r["outT"].T) for r in res.results])
    return out.astype(np.float32)



# revision 25
# speedup vs baseline: 22.9341x; 1.0021x over previous
"""Trainium2 Bass kernel for nn_MultiHeadAttention_84473416778245.

Reference semantics (note two quirks):
  - softmax over the HEAD axis (axis=1), not the key axis -> purely
    elementwise per (q,k): attn[h] = exp(s[h]) / sum_h' exp(s[h'])
  - output reshape [B,H,S,hd] -> [B,S,H*hd] without transpose-back, which is
    a contiguous reinterpretation of the per-batch values buffer.

Sharding: data-parallel over batch B=8 -> one batch element per NeuronCore.

Device dataflow (per core, S=2048, H=8, hd=48, D=384):
  - host feeds x^T (augmented with a ones row for bias), packed transposed
    weights, all fp16
  - QKV projection on PE producing Q^T/K^T in [hd, S] layout (head pairs
    packed at partition bases 0 and 64) and V in [S, 48*H] layout
  - attention: per (q-block 512, k-tile 128):
      scores^T[k,q] per head via row-paired matmuls (K=48 at row groups 0/64)
      exp on ACT (scale fused), head-sum D via DVE adds, G = exp(-ln D) on
      ACT, attn = exp*G on DVE, PV via col-paired matmuls accumulating
      values^T[hd, q] in PSUM
  - out projection: out^T[e, s'] = sum_i W_o^T[48i:48i+48].T @ values^T[:, i::8]
    (the reshape quirk becomes a stride-8 column view), bias via per-partition
    ACT add; out^T DMA'd to HBM; host transposes after gather.
"""

import numpy as np
from contextlib import ExitStack

H, HD, D = 8, 48, 384
S_FULL = 2048
B = 8

_CACHE = {}


def _pack_host(W_qkv, b_qkv, W_o, b_o):
    f16 = np.float16
    wqkT = np.zeros((385, 1024), np.float32)
    for j in range(4):
        hA, hB = 2 * j, 2 * j + 1
        for t, row0 in ((2 * j, 48), (2 * j + 1, 0)):  # K tile, then Q tile
            for col0, h in ((0, hA), (64, hB)):
                rows = slice(144 * h + row0, 144 * h + row0 + 48)
                wqkT[:384, t * 128 + col0 : t * 128 + col0 + 48] = W_qkv[rows, :].T
                wqkT[384, t * 128 + col0 : t * 128 + col0 + 48] = b_qkv[rows]
    wvT = np.zeros((385, 384), np.float32)
    for h in range(H):
        rows = slice(144 * h + 96, 144 * h + 144)
        wvT[:384, 48 * h : 48 * h + 48] = W_qkv[rows, :].T
        wvT[384, 48 * h : 48 * h + 48] = b_qkv[rows]
    woT = np.zeros((128, 8 * 384), np.float32)
    WoT = np.ascontiguousarray(W_o.T)
    for i in range(8):
        woT[0:48, i * 384 : (i + 1) * 384] = WoT[48 * i : 48 * i + 48, :]
        woT[64:112, i * 384 : (i + 1) * 384] = WoT[48 * i : 48 * i + 48, :]
    bo = np.ascontiguousarray(b_o.astype(np.float32).reshape(3, 128, 1))
    qkb = np.zeros((128, 8), np.float32)
    for j in range(4):
        hA, hB = 2 * j, 2 * j + 1
        for t, row0 in ((2 * j, 48), (2 * j + 1, 0)):
            for col0, h in ((0, hA), (64, hB)):
                qkb[col0 : col0 + 48, t] = b_qkv[144 * h + row0 : 144 * h + row0 + 48]
    return wqkT.astype(f16), wvT.astype(f16), woT.astype(f16), bo, qkb


def build_program(S=S_FULL, use_ln_recip=True, repeats=1, ablate=(), d_on_pe=False, exp2048=False, bigbufs=6, v2=True, gp_adds=0, interleave=True, filler=0, g_dve=False):
    """Build the (single-core SPMD) Bass program. Returns compiled nc.

    repeats>1 re-runs the whole compute body serially (same tiles/tags), for
    slope-based HW timing: wall(R) ~ overhead + R * t_kernel."""
    import concourse.bass as bass  # noqa: F401
    import concourse.tile as tile
    from concourse import bacc, mybir

    f16 = mybir.dt.float16
    f32 = mybir.dt.float32
    AF = mybir.ActivationFunctionType

    QB = min(512, S)          # q block
    n_qb = S // QB
    n_kt = S // 128           # k tiles
    n_st = S // 128           # s tiles for V
    SC = min(512, S)          # s chunk for qk^T projection
    n_sc = S // SC
    T = S // 8                # out column block per head
    scale = float(1.0 / np.sqrt(48.0))

    # Force Exp and Ln to resolve to the combined 'natural_log_exp_and_others'
    # ACT table set: the greedy per-function set choice would otherwise
    # alternate exp_and_others <-> natural_log every attention tile (~2.7us
    # per table load). get_activation_tables is functools.cache'd and returns
    # the live dict, so mutate it in place; indices (act_func_set_id) are
    # positional and unchanged.
    from concourse import hw_specs

    _tables = hw_specs.get_activation_tables("gen3")
    for _name, _funcs in _tables.items():
        if _name != "natural_log_exp_and_others":
            _funcs.discard(mybir.ActivationFunctionType.Exp)
            _funcs.discard(mybir.ActivationFunctionType.Ln)

    nc = bacc.Bacc("TRN2", target_bir_lowering=False, debug=False)

    xT_d = nc.dram_tensor("xT", [385, S], f16, kind="ExternalInput").ap()
    wqk_d = nc.dram_tensor("wqkT", [385, 1024], f16, kind="ExternalInput").ap()
    wv_d = nc.dram_tensor("wvT", [385, 384], f16, kind="ExternalInput").ap()
    wo_d = nc.dram_tensor("woT", [128, 3072], f16, kind="ExternalInput").ap()
    qkb_d = nc.dram_tensor("qkb", [128, 8], f32, kind="ExternalInput").ap()
    bo_d = nc.dram_tensor("bo", [3, 128, 1], f32, kind="ExternalInput").ap()
    out_d = nc.dram_tensor("outT", [384, S], f32, kind="ExternalOutput").ap()

    with tile.TileContext(nc) as tc, ExitStack() as ctx:
        const = ctx.enter_context(tc.tile_pool(name="const", bufs=1))
        persist = ctx.enter_context(tc.tile_pool(name="persist", bufs=1))
        big2 = ctx.enter_context(tc.tile_pool(name="big2", bufs=bigbufs))
        small = ctx.enter_context(tc.tile_pool(name="small", bufs=4))
        outp = ctx.enter_context(tc.tile_pool(name="outp", bufs=2))

        # ---- load inputs -------------------------------------------------
        # weights first, then x chunk 0, so the first QKV matmul can start
        # after ~1.2MB of DMA instead of the full input set; x is split into
        # per-chunk tiles so the Tile dep tracker releases chunk 0 early.
        n_ch = S // SC
        wqk = [const.tile([128, 1024], f16, tag=f"wqk{c}", name=f"wqk{c}") for c in range(3)]
        for c in range(3):
            nc.sync.dma_start(wqk[c][:], wqk_d[128 * c : 128 * (c + 1), :])
        wqkch = wqk

        xT = [
            [const.tile([128, SC], f16, tag=f"xT{c}_{k}", name=f"xT{c}_{k}") for k in range(n_ch)]
            for c in range(3)
        ]
        for k in range(n_ch):
            for c in range(3):
                nc.sync.dma_start(
                    xT[c][k][:], xT_d[128 * c : 128 * (c + 1), k * SC : (k + 1) * SC]
                )
        xch = xT

        wv = [const.tile([128, 384], f16, tag=f"wv{c}", name=f"wv{c}") for c in range(3)]
        wv1 = const.tile([1, 384], f16, tag="wv3", name="wv3")
        for c in range(3):
            nc.sync.dma_start(wv[c][:], wv_d[128 * c : 128 * (c + 1), :])
        nc.sync.dma_start(wv1[:], wv_d[384:385, :])
        wvch = wv + [wv1]

        qkb = const.tile([128, 8], f32, tag="qkb", name="qkb")
        nc.sync.dma_start(qkb[:], qkb_d[:, :])
        xones = const.tile([1, S], f16, tag="xones", name="xones")
        nc.vector.memset(xones[:], 1.0)
        wo = const.tile([128, 3072], f16, tag="wo", name="wo")
        nc.sync.dma_start(wo[:], wo_d[:, :])
        bo = [const.tile([128, 1], f32, tag=f"bo{e}", name=f"bo{e}") for e in range(3)]
        for e in range(3):
            nc.sync.dma_start(bo[e][:], bo_d[e])

        # ---- compute body (optionally repeated for slope timing) ---------
        for _rep in range(repeats):
            if v2:
                build_body_v2(nc, tc, mybir, AF, persist, big2, small, outp,
                              xch, wqkch, wvch, wo, bo, out_d,
                              S, QB, n_qb, n_kt, n_st, SC, n_sc, T, scale,
                              f16, f32, gp_adds, interleave, filler, qkb, g_dve, xones)
            else:
                build_body(nc, tc, mybir, AF, persist, big2, small, outp,
                           xch, wqkch, wvch, wo, bo, out_d,
                           S, QB, n_qb, n_kt, n_st, SC, n_sc, T, scale,
                           use_ln_recip, f16, f32, ablate, d_on_pe, exp2048)

    nc.compile()
    return nc


def build_body(nc, tc, mybir, AF, persist, big2, small, outp,
               xch, wqkch, wvch, wo, bo, out_d,
               S, QB, n_qb, n_kt, n_st, SC, n_sc, T, scale,
               use_ln_recip, f16, f32, ablate=(), d_on_pe=False, exp2048=False):
    if True:
        # ---- QKV projection ---------------------------------------------
        qkT = [persist.tile([128, S], f16, tag=f"qkT{t}", name=f"qkT{t}") for t in range(8)]
        V = [persist.tile([128, 384], f16, tag=f"V{st}", name=f"V{st}") for st in range(n_st)]

        with tc.tile_pool(name="qkvps", bufs=2, space="PSUM") as qp:  # per-tile bufs below
            for t in range(8):
                for sc in range(n_sc):
                    ps = qp.tile([128, SC], f32, tag="qk_ps", name="qk_ps", bufs=3)
                    for c in range(4):
                        nc.tensor.matmul(
                            ps[:],
                            wqkch[c][:, t * 128 : (t + 1) * 128],
                            xch[c][:, sc * SC : (sc + 1) * SC],
                            start=(c == 0),
                            stop=(c == 3),
                        )
                    nc.vector.tensor_copy(qkT[t][:, sc * SC : (sc + 1) * SC], ps[:])
            for st in range(n_st):
                ps = qp.tile([128, 384], f32, tag="v_ps", name="v_ps")
                for c in range(4):
                    nc.tensor.matmul(
                        ps[:],
                        xch[c][:, st * 128 : (st + 1) * 128],
                        wvch[c][:],
                        start=(c == 0),
                        stop=(c == 3),
                    )
                nc.scalar.copy(V[st][:], ps[:])

        # ---- attention ---------------------------------------------------
        values = [persist.tile([128, S], f16, tag=f"values{j}", name=f"values{j}") for j in range(4)]

        if d_on_pe:
            from concourse.masks import make_identity

            ident = persist.tile([128, 128], f16, tag="ident", name="ident")
            make_identity(nc, ident[:])

        with (
            tc.tile_pool(name="scps", bufs=(1 if (d_on_pe or exp2048) else 2), space="PSUM") as scp,
            tc.tile_pool(name="pvps", bufs=1, space="PSUM") as pvp,
            tc.tile_pool(name="dps", bufs=2, space="PSUM") as dpp,
        ):
            if "attn" in ablate:
                nc.sync.dma_start(out_d[0:128, 0:192], V[0][:].bitcast(f32))
                return
            for qb in range(n_qb):
                qs = slice(qb * QB, (qb + 1) * QB)
                vps = [pvp.tile([128, QB], f32, tag=f"pv{j}", name=f"pv{j}") for j in range(4)]
                for kt in range(n_kt):
                    ks = slice(kt * 128, (kt + 1) * 128)
                    exp_sb = big2.tile([128, 8 * QB], f16, tag="exp", name="exp_sb")
                    if exp2048:
                        for half in range(2):
                            sps = scp.tile(
                                [128, 2048], f32, tag="sc_ps", name="sc_ps"
                            )
                            for jj in range(2):
                                j = 2 * half + jj
                                nc.tensor.matmul(
                                    sps[:, 1024 * jj : 1024 * jj + QB],
                                    qkT[2 * j][0:48, ks],
                                    qkT[2 * j + 1][0:48, qs],
                                    tile_position=(0, 0),
                                )
                                nc.tensor.matmul(
                                    sps[:, 1024 * jj + 512 : 1024 * jj + 512 + QB],
                                    qkT[2 * j][64:112, ks],
                                    qkT[2 * j + 1][64:112, qs],
                                    tile_position=(64, 0),
                                )
                            sps_v = sps[:, :].rearrange(
                                "p (b q) -> p b q", b=4
                            )[:, :, 0:QB]
                            exp_v = exp_sb[
                                :, 4 * half * QB : 4 * (half + 1) * QB
                            ].rearrange("p (b q) -> p b q", b=4)
                            nc.scalar.activation(exp_v, sps_v, AF.Exp, scale=scale)
                    else:
                        for j in range(4):
                            # each half sits in its own PSUM bank (512 f32 cols)
                            sps = scp.tile([128, 1024], f32, tag="sc_ps", name="sc_ps")
                            nc.tensor.matmul(
                                sps[:, 0:QB],
                                qkT[2 * j][0:48, ks],
                                qkT[2 * j + 1][0:48, qs],
                                tile_position=(0, 0),
                            )
                            nc.tensor.matmul(
                                sps[:, 512 : 512 + QB],
                                qkT[2 * j][64:112, ks],
                                qkT[2 * j + 1][64:112, qs],
                                tile_position=(64, 0),
                            )
                            sps_v = sps[:, :].rearrange("p (b q) -> p b q", b=2)[
                                :, :, 0:QB
                            ]
                            exp_v = exp_sb[
                                :, 2 * j * QB : (2 * j + 2) * QB
                            ].rearrange("p (b q) -> p b q", b=2)
                            nc.scalar.activation(exp_v, sps_v, AF.Exp, scale=scale)
                    if "norm" in ablate:
                        attn = exp_sb
                    else:
                        attn = None
                    if attn is None and d_on_pe:
                        D_ps = dpp.tile([128, QB], f32, tag="D_ps", name="D_ps")
                        for h in range(8):
                            nc.tensor.matmul(
                                D_ps[:],
                                ident[:],
                                exp_sb[:, h * QB : (h + 1) * QB],
                                start=(h == 0),
                                stop=(h == 7),
                            )
                        Gt = small.tile([128, QB], f16, tag="G", name="Gt")
                        lnD = small.tile([128, QB], f16, tag="lnD", name="lnD")
                        nc.scalar.activation(lnD[:], D_ps[:], AF.Ln)
                        nc.scalar.activation(Gt[:], lnD[:], AF.Exp, scale=-1.0)
                    # D = sum over heads: 4-op tree; first two ops start as
                    # soon as exp waves 1 and 3 land (better pipelining)
                    elif attn is None:
                        D2 = small.tile([128, 4 * QB], f16, tag="D2", name="D2")
                        nc.vector.tensor_add(
                            D2[:, 0 : 2 * QB],
                            exp_sb[:, 0 : 2 * QB],
                            exp_sb[:, 2 * QB : 4 * QB],
                        )
                        nc.vector.tensor_add(
                            D2[:, 2 * QB : 4 * QB],
                            exp_sb[:, 4 * QB : 6 * QB],
                            exp_sb[:, 6 * QB : 8 * QB],
                        )
                        nc.vector.tensor_add(
                            D2[:, 0 : 2 * QB], D2[:, 0 : 2 * QB], D2[:, 2 * QB : 4 * QB]
                        )
                        Dt = small.tile([128, QB], f16, tag="D", name="Dt")
                        nc.vector.tensor_add(Dt[:], D2[:, 0:QB], D2[:, QB : 2 * QB])
                        Gt = small.tile([128, QB], f16, tag="G", name="Gt")
                        if use_ln_recip:
                            lnD = small.tile([128, QB], f16, tag="lnD", name="lnD")
                            nc.scalar.activation(lnD[:], Dt[:], AF.Ln)
                            nc.scalar.activation(Gt[:], lnD[:], AF.Exp, scale=-1.0)
                        else:
                            Df = small.tile([128, QB], f32, tag="Df", name="Df")
                            nc.vector.tensor_copy(Df[:], Dt[:])
                            Gf = small.tile([128, QB], f32, tag="Gf", name="Gf")
                            nc.vector.reciprocal_approx_fast(Gf[:], Df[:])
                            nc.vector.tensor_copy(Gt[:], Gf[:])
                    if attn is None:
                        attn = big2.tile([128, 8 * QB], f16, tag="attn", name="attn")
                        # fused muls: 2 ops of 4 heads each; G broadcast via a
                        # step-0 middle AP dim (innermost stays step-1 so the
                        # DVE 2x_1p mode is preserved)
                        g_b = Gt[:].rearrange("p (o q) -> p o q", o=1).broadcast_to([128, 4, QB])
                        for w in range(2):
                            nc.vector.tensor_mul(
                                attn[:, 4 * w * QB : 4 * (w + 1) * QB].rearrange(
                                    "p (h q) -> p h q", h=4
                                ),
                                exp_sb[:, 4 * w * QB : 4 * (w + 1) * QB].rearrange(
                                    "p (h q) -> p h q", h=4
                                ),
                                g_b,
                            )
                    if "pv" in ablate:
                        if kt == n_kt - 1:
                            nc.sync.dma_start(
                                out_d[0:128, :],
                                attn[:, 0 : 8 * QB].bitcast(f32)[:, 0 : S],
                            )
                        continue
                    for j in range(4):
                        nc.tensor.matmul(
                            vps[j][0:48, :],
                            V[kt][:, 96 * j : 96 * j + 48],
                            attn[:, 2 * j * QB : (2 * j + 1) * QB],
                            start=(kt == 0),
                            stop=(kt == n_kt - 1),
                            tile_position=(0, 0),
                        )
                        nc.tensor.matmul(
                            vps[j][64:112, :],
                            V[kt][:, 96 * j + 48 : 96 * j + 96],
                            attn[:, (2 * j + 1) * QB : (2 * j + 2) * QB],
                            start=(kt == 0),
                            stop=(kt == n_kt - 1),
                            tile_position=(0, 64),
                            # disjoint partition range (64:112) of the same
                            # bank as the (0,0) group; group check is
                            # bank-granular and would false-positive
                            skip_group_check=True,
                        )
                if "pv" in ablate:
                    continue
                for j in range(4):
                    nc.vector.tensor_copy(values[j][0:48, qs], vps[j][0:48, :])
                    nc.vector.tensor_copy(
                        values[j][64:112, qs], vps[j][64:112, :]
                    )

        if "pv" in ablate:
            return
        # ---- output projection -------------------------------------------
        # Each head owns a full 512-f32-col PSUM region (start=True lazily
        # zeroes the whole region), 4 heads per [128, 2048] tile, two halves
        # per e-tile. Even heads read values partitions 0:48 (row group 0),
        # odd heads partitions 64:112 (row group 64, duplicated W_o^T rows).
        with tc.tile_pool(name="outps", bufs=2, space="PSUM") as op:
            for et in range(3):
                osb = outp.tile([128, S], f32, tag="osb", name="osb")
                for half in range(2):
                    ops_ = op.tile([128, 2048], f32, tag="out_ps", name="out_ps")
                    for hh in range(4):
                        h = 4 * half + hh
                        base = 0 if h % 2 == 0 else 64
                        rhs = values[h // 2][base : base + 48, :].rearrange(
                            "p (t i) -> p i t", i=8
                        )
                        for i in range(8):
                            nc.tensor.matmul(
                                ops_[:, 512 * hh : 512 * hh + T],
                                wo[
                                    base : base + 48,
                                    i * 384 + et * 128 : i * 384 + et * 128 + 128,
                                ],
                                rhs[:, i, :],
                                start=(i == 0),
                                stop=(i == 7),
                                tile_position=(base, 0),
                            )
                    ops_v = ops_[:, :].rearrange("p (hh q) -> p hh q", hh=4)[
                        :, :, 0:T
                    ]
                    osb_v = osb[
                        :, 4 * half * T : 4 * (half + 1) * T
                    ].rearrange("p (hh q) -> p hh q", hh=4)
                    nc.scalar.activation(
                        osb_v, ops_v, AF.Identity, bias=bo[et][:]
                    )
                nc.sync.dma_start(out_d[et * 128 : (et + 1) * 128, :], osb[:])


def build_body_v2(nc, tc, mybir, AF, persist, big2, small, outp,
                  xch, wqkch, wvch, wo, bo, out_d,
                  S, QB, n_qb, n_kt, n_st, SC, n_sc, T, scale,
                  f16, f32, gp_adds=1, interleave=True, filler=0, qkb=None, g_dve=False, xones=None):
    from contextlib import ExitStack as _ES
    _fes = _ES()

    def emit_filler(n):
        # dummy weight loads: pure PE-array activity (no PSUM write, own
        # SBUF read port) to keep the HAM activity monitor from
        # re-throttling the PE clock during ACT/DVE-paced stretches.
        for _ in range(n):
            nc.tensor.ldweights(wo[0:128, 0:512])
    """Rebalanced attention pipeline:
      - one D-tree add offloaded to GPSIMD (Pool) to unload DVE
      - values stored head-interleaved (i-major) so the out-projection
        streams a contiguous rhs instead of a stride-8 view
      - out-projection per-head rhs is then values[j][base:, i*T:(i+1)*T]
    """
    # ---- QKV projection + attention (interleaved) --------------------
    qkT = [persist.tile([128, S], f16, tag=f"qkT{t}", name=f"qkT{t}") for t in range(8)]
    V = [persist.tile([128, 384], f16, tag=f"V{st}", name=f"V{st}") for st in range(n_st)]
    values = [persist.tile([128, S], f16, tag=f"values{j}", name=f"values{j}") for j in range(4)]

    with tc.tile_pool(name="qkvps", bufs=2, space="PSUM") as qp:
        for sc in range(n_sc):
            for t in range(8):
                ps = qp.tile([128, SC], f32, tag="qk_ps", name="qk_ps", bufs=3)
                for c in range(3):
                    nc.tensor.matmul(
                        ps[:],
                        wqkch[c][:, t * 128 : (t + 1) * 128],
                        xch[c][sc][:],
                        start=(c == 0),
                        stop=(c == 2),
                    )
                # Q/K bias is per-partition in this layout: fold it into the
                # PSUM->SBUF copy instead of a 4th (1-row) matmul chunk.
                nc.vector.tensor_scalar_add(
                    qkT[t][:, sc * SC : (sc + 1) * SC], ps[:],
                    qkb[:, t : t + 1],
                )
            for st in range(sc * n_st // n_sc, (sc + 1) * n_st // n_sc):
                ps = qp.tile([128, 384], f32, tag="v_ps", name="v_ps")
                kk, off = st // (n_st // n_sc), (st % (n_st // n_sc)) * 128
                for c in range(4):
                    nc.tensor.matmul(
                        ps[:],
                        xones[:, st * 128 : (st + 1) * 128]
                        if c == 3
                        else xch[c][kk][:, off : off + 128],
                        wvch[c][:],
                        start=(c == 0),
                        stop=(c == 3),
                    )
                nc.scalar.copy(V[st][:], ps[:])

    with (
        tc.tile_pool(name="scps", bufs=2, space="PSUM") as scp,
        tc.tile_pool(name="pvps", bufs=1, space="PSUM") as pvp,
    ):
        for qb in range(n_qb):
            qs = slice(qb * QB, (qb + 1) * QB)
            vps = [pvp.tile([128, QB], f32, tag=f"pv{j}", name=f"pv{j}") for j in range(4)]

            def emit_pv(attn_t, kt):
                for j in range(4):
                    nc.tensor.matmul(
                        vps[j][0:48, :],
                        V[kt][:, 96 * j : 96 * j + 48],
                        attn_t[:, 2 * j * QB : (2 * j + 1) * QB],
                        start=(kt == 0),
                        stop=(kt == n_kt - 1),
                        tile_position=(0, 0),
                    )
                    nc.tensor.matmul(
                        vps[j][64:112, :],
                        V[kt][:, 96 * j + 48 : 96 * j + 96],
                        attn_t[:, (2 * j + 1) * QB : (2 * j + 2) * QB],
                        start=(kt == 0),
                        stop=(kt == n_kt - 1),
                        tile_position=(0, 64),
                        skip_group_check=True,
                    )
                    if filler:
                        emit_filler(filler)

            # Software pipeline, per iteration kt:
            #   PE:  scores(kt), then PV(kt-1) -- PV issued after the next
            #        tile's score matmuls so the PE's in-order queue isn't
            #        head-of-line blocked on kt-1's normalized weights
            #   ACT: exps(kt), then Ln/ExpG(kt-1)
            #   DVE: D-tree adds(kt), then in-place normalize muls(kt-1)
            # (Deeper deferral measures WORSE: longer PE idle stretches
            # trigger more HAM clock re-throttles.)
            pending = None
            for kt in range(n_kt):
                if True:
                    ks = slice(kt * 128, (kt + 1) * 128)
                    exp_sb = big2.tile([128, 8 * QB], f16, tag="exp", name="exp_sb")
                    for j in range(4):
                        # each half sits in its own PSUM bank (512 f32 cols)
                        sps = scp.tile([128, 1024], f32, tag="sc_ps", name="sc_ps")
                        nc.tensor.matmul(
                            sps[:, 0:QB],
                            qkT[2 * j][0:48, ks],
                            qkT[2 * j + 1][0:48, qs],
                            tile_position=(0, 0),
                        )
                        nc.tensor.matmul(
                            sps[:, 512 : 512 + QB],
                            qkT[2 * j][64:112, ks],
                            qkT[2 * j + 1][64:112, qs],
                            tile_position=(64, 0),
                        )
                        sps_v = sps[:, :].rearrange("p (b q) -> p b q", b=2)[
                            :, :, 0:QB
                        ]
                        exp_v = exp_sb[
                            :, 2 * j * QB : (2 * j + 2) * QB
                        ].rearrange("p (h q) -> p h q", h=2)
                        nc.scalar.activation(exp_v, sps_v, AF.Exp, scale=scale)
                # PV for the previous tile goes here, after this tile's
                # score matmuls, so the PE's in-order queue isn't blocked
                if pending is not None:
                    emit_pv(*pending)
                # D = sum over heads: 4-op tree
                D2 = small.tile([128, 4 * QB], f16, tag="D2", name="D2")
                nc.vector.tensor_add(
                    D2[:, 0 : 2 * QB],
                    exp_sb[:, 0 : 2 * QB],
                    exp_sb[:, 2 * QB : 4 * QB],
                )
                nc.vector.tensor_add(
                    D2[:, 2 * QB : 4 * QB],
                    exp_sb[:, 4 * QB : 6 * QB],
                    exp_sb[:, 6 * QB : 8 * QB],
                )
                nc.vector.tensor_add(
                    D2[:, 0 : 2 * QB], D2[:, 0 : 2 * QB], D2[:, 2 * QB : 4 * QB]
                )
                Dt = small.tile([128, QB], f16, tag="D", name="Dt")
                nc.vector.tensor_add(Dt[:], D2[:, 0:QB], D2[:, QB : 2 * QB])
                Gt = small.tile([128, QB], f16, tag="G", name="Gt")
                lnD = small.tile([128, QB], f16, tag="lnD", name="lnD")
                nc.scalar.activation(lnD[:], Dt[:], AF.Ln)
                nc.scalar.activation(Gt[:], lnD[:], AF.Exp, scale=-1.0)
                g_b = Gt[:].rearrange("p (o q) -> p o q", o=1).broadcast_to(
                    [128, 4, QB]
                )
                for w in range(2):
                    # in place: DVE is in-order, the D-tree adds above
                    # already read these columns
                    nc.vector.tensor_mul(
                        exp_sb[:, 4 * w * QB : 4 * (w + 1) * QB].rearrange(
                            "p (h q) -> p h q", h=4
                        ),
                        exp_sb[:, 4 * w * QB : 4 * (w + 1) * QB].rearrange(
                            "p (h q) -> p h q", h=4
                        ),
                        g_b,
                    )
                pending = (exp_sb, kt)
            emit_pv(*pending)
            # unload PV accumulators. With interleave=True, scatter the q
            # columns i-major (col i*T + t holds q=8t+i) so the out
            # projection's per-i rhs blocks are contiguous.
            U = QB // 8
            for j in range(4):
                for base in (0, 64):
                    src = vps[j][base : base + 48, :]
                    if interleave:
                        src_v = src.rearrange("p (u i) -> p i u", i=8)
                        dst_v = (
                            values[j][base : base + 48, :]
                            .rearrange("p (i t) -> p i t", i=8)[
                                :, :, qb * U : (qb + 1) * U
                            ]
                        )
                        nc.vector.tensor_copy(dst_v, src_v)
                    else:
                        nc.vector.tensor_copy(
                            values[j][base : base + 48, qs], src
                        )

    # ---- output projection -------------------------------------------
    with tc.tile_pool(name="outps", bufs=2, space="PSUM") as op:
        for et in range(3):
            osb = outp.tile([128, S], f32, tag="osb", name="osb")
            for half in range(2):
                # each head owns a full 512-f32-col PSUM region: accumulation
                # group zeroing is bank-granular
                ops_ = op.tile([128, 2048], f32, tag="out_ps", name="out_ps")
                for hh in range(4):
                    h = 4 * half + hh
                    base = 0 if h % 2 == 0 else 64
                    vt = values[h // 2]
                    for i in range(8):
                        if interleave:
                            rhs = vt[base : base + 48, i * T : (i + 1) * T]
                        else:
                            rhs = vt[base : base + 48, :].rearrange(
                                "p (t i) -> p i t", i=8
                            )[:, i, :]
                        nc.tensor.matmul(
                            ops_[:, 512 * hh : 512 * hh + T],
                            wo[
                                base : base + 48,
                                i * 384 + et * 128 : i * 384 + et * 128 + 128,
                            ],
                            rhs,
                            start=(i == 0),
                            stop=(i == 7),
                            tile_position=(base, 0),
                        )
                ops_v = ops_[:, :].rearrange("p (hh q) -> p hh q", hh=4)[
                    :, :, 0:T
                ]
                osb_v = osb[
                    :, 4 * half * T : 4 * (half + 1) * T
                ].rearrange("p (hh q) -> p hh q", hh=4)
                nc.scalar.activation(
                    osb_v, ops_v, AF.Identity, bias=bo[et][:]
                )
            nc.sync.dma_start(out_d[et * 128 : (et + 1) * 128, :], osb[:])


def _get_program(S=S_FULL):
    key = ("nc", S)
    if key not in _CACHE:
        _CACHE[key] = build_program(S)
    return _CACHE[key]


def kernel(x, W_qkv, b_qkv, W_o, b_o):
    from concourse import bass_utils

    x = np.asarray(x, dtype=np.float32)
    W_qkv = np.asarray(W_qkv, dtype=np.float32)
    b_qkv = np.asarray(b_qkv, dtype=np.float32)
    W_o = np.asarray(W_o, dtype=np.float32)
    b_o = np.asarray(b_o, dtype=np.float32)
    Bx, S, _ = x.shape

    wqkT, wvT, woT, bo, qkb = _pack_host(W_qkv, b_qkv, W_o, b_o)
    in_maps = []
    for b in range(Bx):
        xT = np.ones((385, S), np.float32)
        xT[:384] = x[b].T
        in_maps.append(
            {
                "xT": xT.astype(np.float16),
                "wqkT": wqkT,
                "wvT": wvT,
                "woT": woT,
                "bo": bo,
                "qkb": qkb,
            }
        )

    nc = _get_program(S)
    res = bass_utils.run_bass_kernel_spmd(nc, in_maps, core_ids=list(range(Bx)))
    out = np.stack([np.ascontiguousarray(r["outT"].T) for r in res.results])
    return out.astype(np.float32)



# revision 29
# speedup vs baseline: 23.2146x; 1.0122x over previous
"""Trainium2 Bass kernel for nn_MultiHeadAttention_84473416778245.

Reference semantics (note two quirks):
  - softmax over the HEAD axis (axis=1), not the key axis -> purely
    elementwise per (q,k): attn[h] = exp(s[h]) / sum_h' exp(s[h'])
  - output reshape [B,H,S,hd] -> [B,S,H*hd] without transpose-back, which is
    a contiguous reinterpretation of the per-batch values buffer.

Sharding: data-parallel over batch B=8 -> one batch element per NeuronCore.

Device dataflow (per core, S=2048, H=8, hd=48, D=384):
  - host feeds x^T (per-512-chunk tiles so compute starts early), packed
    transposed weights, all fp16; Q/K bias folded into the PSUM->SBUF copy
    (tensor_scalar_add with a per-partition bias column) instead of a 4th
    1-row matmul chunk
  - QKV projection on PE producing Q^T/K^T in [hd, S] layout (head pairs
    packed at partition bases 0 and 64) and V in [S, 48*H] layout
  - attention: per (q-block 512, k-tile 128), software-pipelined one tile:
      scores^T[k,q] per head via row-paired matmuls (K=48 at row groups
      0/64); PV of tile kt-1 is emitted AFTER tile kt's score matmuls so
      the PE's in-order queue is never head-of-line blocked on the
      normalize chain; exp on ACT (scale fused) from PSUM; head-sum D via
      DVE add tree; G = exp(-ln D) on ACT (Exp+Ln share one ACT table
      set, forced at build); normalize IN PLACE on exp_sb (DVE in-order
      guarantees the adds read first); PV via col-paired matmuls
      accumulating values^T[hd, q] in PSUM
  - values are unloaded PSUM->SBUF with an i-major interleaved column
    scatter (col i*256+t holds q=8t+i), which turns the out-projection's
    reshape-quirk stride-8 rhs into contiguous [48, 256] blocks
  - out projection: out^T[e, s] = sum_i W_o blocks @ values[i-block], bias
    via per-partition ACT add; out^T DMA'd to HBM; host transposes.

Perf notes (measured on axon trn2, NTFF traces):
  - device span 488 us vs 642 us for the previous version; engines end up
    3-way balanced: ACT ~341 us (exp floor is 255 us: 262144 elems/
    partition at ~1.2 GHz, 1 elem/cycle/lane), PE ~320 us, DVE ~288 us.
  - the PE runs mostly clock-throttled at 1.2 GHz (HAM): any ~1 us PE
    idle gap re-throttles, and un-throttling needs ~3.4 us of SUSTAINED
    PE activity, which a balanced 3-engine kernel never produces. Deeper
    software pipelining (PV deferred 2-3 tiles, batched G) measured WORSE
    (longer PE idle stretches -> more re-throttles). Dummy-LDWEIGHTS
    filler to keep HAM warm fails walrus codegen (standalone InstLdweights
    unsupported). fp8 doesn't help: PE throughput is column-paced and the
    attention matmuls are column-bound (K=48/128), not contraction-bound.
  - PSUM is the binding resource: 2x2-bank score ring + 4 banks of PV
    accumulators = all 8 banks, which caps exp calls at N=1024 and the
    scores lookahead at ~1 tile.
"""

import numpy as np
from contextlib import ExitStack

H, HD, D = 8, 48, 384
S_FULL = 2048
B = 8

_CACHE = {}


def _pack_host(W_qkv, b_qkv, W_o, b_o):
    f16 = np.float16
    wqkT = np.zeros((385, 1024), np.float32)
    for j in range(4):
        hA, hB = 2 * j, 2 * j + 1
        for t, row0 in ((2 * j, 48), (2 * j + 1, 0)):  # K tile, then Q tile
            for col0, h in ((0, hA), (64, hB)):
                rows = slice(144 * h + row0, 144 * h + row0 + 48)
                wqkT[:384, t * 128 + col0 : t * 128 + col0 + 48] = W_qkv[rows, :].T
                wqkT[384, t * 128 + col0 : t * 128 + col0 + 48] = b_qkv[rows]
    wvT = np.zeros((385, 384), np.float32)
    for h in range(H):
        rows = slice(144 * h + 96, 144 * h + 144)
        wvT[:384, 48 * h : 48 * h + 48] = W_qkv[rows, :].T
        wvT[384, 48 * h : 48 * h + 48] = b_qkv[rows]
    woT = np.zeros((128, 8 * 384), np.float32)
    WoT = np.ascontiguousarray(W_o.T)
    for i in range(8):
        woT[0:48, i * 384 : (i + 1) * 384] = WoT[48 * i : 48 * i + 48, :]
        woT[64:112, i * 384 : (i + 1) * 384] = WoT[48 * i : 48 * i + 48, :]
    bo = np.ascontiguousarray(b_o.astype(np.float32).reshape(3, 128, 1))
    qkb = np.zeros((128, 8), np.float32)
    for j in range(4):
        hA, hB = 2 * j, 2 * j + 1
        for t, row0 in ((2 * j, 48), (2 * j + 1, 0)):
            for col0, h in ((0, hA), (64, hB)):
                qkb[col0 : col0 + 48, t] = b_qkv[144 * h + row0 : 144 * h + row0 + 48]
    return wqkT.astype(f16), wvT.astype(f16), woT.astype(f16), bo, qkb


def build_program(S=S_FULL, use_ln_recip=True, repeats=1, ablate=(), d_on_pe=False, exp2048=False, bigbufs=6, v2=True, gp_adds=0, interleave=True, filler=0, g_dve=False):
    """Build the (single-core SPMD) Bass program. Returns compiled nc.

    repeats>1 re-runs the whole compute body serially (same tiles/tags), for
    slope-based HW timing: wall(R) ~ overhead + R * t_kernel."""
    import concourse.bass as bass  # noqa: F401
    import concourse.tile as tile
    from concourse import bacc, mybir

    f16 = mybir.dt.float16
    f32 = mybir.dt.float32
    AF = mybir.ActivationFunctionType

    QB = min(512, S)          # q block
    n_qb = S // QB
    n_kt = S // 128           # k tiles
    n_st = S // 128           # s tiles for V
    SC = min(512, S)          # s chunk for qk^T projection
    n_sc = S // SC
    T = S // 8                # out column block per head
    scale = float(1.0 / np.sqrt(48.0))

    # Force Exp and Ln to resolve to the combined 'natural_log_exp_and_others'
    # ACT table set: the greedy per-function set choice would otherwise
    # alternate exp_and_others <-> natural_log every attention tile (~2.7us
    # per table load). get_activation_tables is functools.cache'd and returns
    # the live dict, so mutate it in place; indices (act_func_set_id) are
    # positional and unchanged.
    from concourse import hw_specs

    _tables = hw_specs.get_activation_tables("gen3")
    for _name, _funcs in _tables.items():
        if _name != "natural_log_exp_and_others":
            _funcs.discard(mybir.ActivationFunctionType.Exp)
            _funcs.discard(mybir.ActivationFunctionType.Ln)

    nc = bacc.Bacc("TRN2", target_bir_lowering=False, debug=False)

    xT_d = nc.dram_tensor("xT", [385, S], f16, kind="ExternalInput").ap()
    wqk_d = nc.dram_tensor("wqkT", [385, 1024], f16, kind="ExternalInput").ap()
    wv_d = nc.dram_tensor("wvT", [385, 384], f16, kind="ExternalInput").ap()
    wo_d = nc.dram_tensor("woT", [128, 3072], f16, kind="ExternalInput").ap()
    qkb_d = nc.dram_tensor("qkb", [128, 8], f32, kind="ExternalInput").ap()
    bo_d = nc.dram_tensor("bo", [3, 128, 1], f32, kind="ExternalInput").ap()
    out_d = nc.dram_tensor("outT", [384, S], f32, kind="ExternalOutput").ap()

    with tile.TileContext(nc) as tc, ExitStack() as ctx:
        const = ctx.enter_context(tc.tile_pool(name="const", bufs=1))
        persist = ctx.enter_context(tc.tile_pool(name="persist", bufs=1))
        big2 = ctx.enter_context(tc.tile_pool(name="big2", bufs=bigbufs))
        small = ctx.enter_context(tc.tile_pool(name="small", bufs=6))
        outp = ctx.enter_context(tc.tile_pool(name="outp", bufs=2))

        # ---- load inputs -------------------------------------------------
        xT = [const.tile([128, S], f16, tag=f"xT{c}", name=f"xT{c}") for c in range(3)]
        for c in range(3):
            nc.sync.dma_start(xT[c][:], xT_d[128 * c : 128 * (c + 1), :])
        xch = xT

        wqk = [const.tile([128, 1024], f16, tag=f"wqk{c}", name=f"wqk{c}") for c in range(3)]
        for c in range(3):
            nc.sync.dma_start(wqk[c][:], wqk_d[128 * c : 128 * (c + 1), :])
        wqkch = wqk

        wv = [const.tile([128, 384], f16, tag=f"wv{c}", name=f"wv{c}") for c in range(3)]
        wv1 = const.tile([1, 384], f16, tag="wv3", name="wv3")
        for c in range(3):
            nc.sync.dma_start(wv[c][:], wv_d[128 * c : 128 * (c + 1), :])
        nc.sync.dma_start(wv1[:], wv_d[384:385, :])
        wvch = wv + [wv1]

        qkb = const.tile([128, 8], f32, tag="qkb", name="qkb")
        nc.sync.dma_start(qkb[:], qkb_d[:, :])
        xones = const.tile([1, S], f16, tag="xones", name="xones")
        nc.vector.memset(xones[:], 1.0)
        wo = const.tile([128, 3072], f16, tag="wo", name="wo")
        nc.sync.dma_start(wo[:], wo_d[:, :])
        bo = [const.tile([128, 1], f32, tag=f"bo{e}", name=f"bo{e}") for e in range(3)]
        for e in range(3):
            nc.sync.dma_start(bo[e][:], bo_d[e])

        # ---- compute body (optionally repeated for slope timing) ---------
        for _rep in range(repeats):
            if v2:
                build_body_v2(nc, tc, mybir, AF, persist, big2, small, outp,
                              xch, wqkch, wvch, wo, bo, out_d,
                              S, QB, n_qb, n_kt, n_st, SC, n_sc, T, scale,
                              f16, f32, gp_adds, interleave, filler, qkb, g_dve, xones)
            else:
                build_body(nc, tc, mybir, AF, persist, big2, small, outp,
                           xch, wqkch, wvch, wo, bo, out_d,
                           S, QB, n_qb, n_kt, n_st, SC, n_sc, T, scale,
                           use_ln_recip, f16, f32, ablate, d_on_pe, exp2048)

    nc.compile()
    return nc


def build_body(nc, tc, mybir, AF, persist, big2, small, outp,
               xch, wqkch, wvch, wo, bo, out_d,
               S, QB, n_qb, n_kt, n_st, SC, n_sc, T, scale,
               use_ln_recip, f16, f32, ablate=(), d_on_pe=False, exp2048=False):
    if True:
        # ---- QKV projection ---------------------------------------------
        qkT = [persist.tile([128, S], f16, tag=f"qkT{t}", name=f"qkT{t}") for t in range(8)]
        V = [persist.tile([128, 384], f16, tag=f"V{st}", name=f"V{st}") for st in range(n_st)]

        with tc.tile_pool(name="qkvps", bufs=2, space="PSUM") as qp:  # per-tile bufs below
            for t in range(8):
                for sc in range(n_sc):
                    ps = qp.tile([128, SC], f32, tag="qk_ps", name="qk_ps", bufs=3)
                    for c in range(4):
                        nc.tensor.matmul(
                            ps[:],
                            wqkch[c][:, t * 128 : (t + 1) * 128],
                            xch[c][:, sc * SC : (sc + 1) * SC],
                            start=(c == 0),
                            stop=(c == 3),
                        )
                    nc.vector.tensor_copy(qkT[t][:, sc * SC : (sc + 1) * SC], ps[:])
            for st in range(n_st):
                ps = qp.tile([128, 384], f32, tag="v_ps", name="v_ps")
                for c in range(4):
                    nc.tensor.matmul(
                        ps[:],
                        xch[c][:, st * 128 : (st + 1) * 128],
                        wvch[c][:],
                        start=(c == 0),
                        stop=(c == 3),
                    )
                nc.vector.tensor_copy(V[st][:], ps[:])

        # ---- attention ---------------------------------------------------
        values = [persist.tile([128, S], f16, tag=f"values{j}", name=f"values{j}") for j in range(4)]

        if d_on_pe:
            from concourse.masks import make_identity

            ident = persist.tile([128, 128], f16, tag="ident", name="ident")
            make_identity(nc, ident[:])

        with (
            tc.tile_pool(name="scps", bufs=(1 if (d_on_pe or exp2048) else 2), space="PSUM") as scp,
            tc.tile_pool(name="pvps", bufs=1, space="PSUM") as pvp,
            tc.tile_pool(name="dps", bufs=2, space="PSUM") as dpp,
        ):
            if "attn" in ablate:
                nc.sync.dma_start(out_d[0:128, 0:192], V[0][:].bitcast(f32))
                return
            for qb in range(n_qb):
                qs = slice(qb * QB, (qb + 1) * QB)
                vps = [pvp.tile([128, QB], f32, tag=f"pv{j}", name=f"pv{j}") for j in range(4)]
                for kt in range(n_kt):
                    ks = slice(kt * 128, (kt + 1) * 128)
                    exp_sb = big2.tile([128, 8 * QB], f16, tag="exp", name="exp_sb")
                    if exp2048:
                        for half in range(2):
                            sps = scp.tile(
                                [128, 2048], f32, tag="sc_ps", name="sc_ps"
                            )
                            for jj in range(2):
                                j = 2 * half + jj
                                nc.tensor.matmul(
                                    sps[:, 1024 * jj : 1024 * jj + QB],
                                    qkT[2 * j][0:48, ks],
                                    qkT[2 * j + 1][0:48, qs],
                                    tile_position=(0, 0),
                                )
                                nc.tensor.matmul(
                                    sps[:, 1024 * jj + 512 : 1024 * jj + 512 + QB],
                                    qkT[2 * j][64:112, ks],
                                    qkT[2 * j + 1][64:112, qs],
                                    tile_position=(64, 0),
                                )
                            sps_v = sps[:, :].rearrange(
                                "p (b q) -> p b q", b=4
                            )[:, :, 0:QB]
                            exp_v = exp_sb[
                                :, 4 * half * QB : 4 * (half + 1) * QB
                            ].rearrange("p (b q) -> p b q", b=4)
                            nc.scalar.activation(exp_v, sps_v, AF.Exp, scale=scale)
                    else:
                        for j in range(4):
                            # each half sits in its own PSUM bank (512 f32 cols)
                            sps = scp.tile([128, 1024], f32, tag="sc_ps", name="sc_ps")
                            nc.tensor.matmul(
                                sps[:, 0:QB],
                                qkT[2 * j][0:48, ks],
                                qkT[2 * j + 1][0:48, qs],
                                tile_position=(0, 0),
                            )
                            nc.tensor.matmul(
                                sps[:, 512 : 512 + QB],
                                qkT[2 * j][64:112, ks],
                                qkT[2 * j + 1][64:112, qs],
                                tile_position=(64, 0),
                            )
                            sps_v = sps[:, :].rearrange("p (b q) -> p b q", b=2)[
                                :, :, 0:QB
                            ]
                            exp_v = exp_sb[
                                :, 2 * j * QB : (2 * j + 2) * QB
                            ].rearrange("p (b q) -> p b q", b=2)
                            nc.scalar.activation(exp_v, sps_v, AF.Exp, scale=scale)
                    if "norm" in ablate:
                        attn = exp_sb
                    else:
                        attn = None
                    if attn is None and d_on_pe:
                        D_ps = dpp.tile([128, QB], f32, tag="D_ps", name="D_ps")
                        for h in range(8):
                            nc.tensor.matmul(
                                D_ps[:],
                                ident[:],
                                exp_sb[:, h * QB : (h + 1) * QB],
                                start=(h == 0),
                                stop=(h == 7),
                            )
                        Gt = small.tile([128, QB], f16, tag="G", name="Gt")
                        lnD = small.tile([128, QB], f16, tag="lnD", name="lnD")
                        nc.scalar.activation(lnD[:], D_ps[:], AF.Ln)
                        nc.scalar.activation(Gt[:], lnD[:], AF.Exp, scale=-1.0)
                    # D = sum over heads: 4-op tree; first two ops start as
                    # soon as exp waves 1 and 3 land (better pipelining)
                    elif attn is None:
                        D2 = small.tile([128, 4 * QB], f16, tag="D2", name="D2")
                        nc.vector.tensor_add(
                            D2[:, 0 : 2 * QB],
                            exp_sb[:, 0 : 2 * QB],
                            exp_sb[:, 2 * QB : 4 * QB],
                        )
                        nc.vector.tensor_add(
                            D2[:, 2 * QB : 4 * QB],
                            exp_sb[:, 4 * QB : 6 * QB],
                            exp_sb[:, 6 * QB : 8 * QB],
                        )
                        nc.vector.tensor_add(
                            D2[:, 0 : 2 * QB], D2[:, 0 : 2 * QB], D2[:, 2 * QB : 4 * QB]
                        )
                        Dt = small.tile([128, QB], f16, tag="D", name="Dt")
                        nc.vector.tensor_add(Dt[:], D2[:, 0:QB], D2[:, QB : 2 * QB])
                        Gt = small.tile([128, QB], f16, tag="G", name="Gt")
                        if use_ln_recip:
                            lnD = small.tile([128, QB], f16, tag="lnD", name="lnD")
                            nc.scalar.activation(lnD[:], Dt[:], AF.Ln)
                            nc.scalar.activation(Gt[:], lnD[:], AF.Exp, scale=-1.0)
                        else:
                            Df = small.tile([128, QB], f32, tag="Df", name="Df")
                            nc.vector.tensor_copy(Df[:], Dt[:])
                            Gf = small.tile([128, QB], f32, tag="Gf", name="Gf")
                            nc.vector.reciprocal_approx_fast(Gf[:], Df[:])
                            nc.vector.tensor_copy(Gt[:], Gf[:])
                    if attn is None:
                        attn = big2.tile([128, 8 * QB], f16, tag="attn", name="attn")
                        # fused muls: 2 ops of 4 heads each; G broadcast via a
                        # step-0 middle AP dim (innermost stays step-1 so the
                        # DVE 2x_1p mode is preserved)
                        g_b = Gt[:].rearrange("p (o q) -> p o q", o=1).broadcast_to([128, 4, QB])
                        for w in range(2):
                            nc.vector.tensor_mul(
                                attn[:, 4 * w * QB : 4 * (w + 1) * QB].rearrange(
                                    "p (h q) -> p h q", h=4
                                ),
                                exp_sb[:, 4 * w * QB : 4 * (w + 1) * QB].rearrange(
                                    "p (h q) -> p h q", h=4
                                ),
                                g_b,
                            )
                    if "pv" in ablate:
                        if kt == n_kt - 1:
                            nc.sync.dma_start(
                                out_d[0:128, :],
                                attn[:, 0 : 8 * QB].bitcast(f32)[:, 0 : S],
                            )
                        continue
                    for j in range(4):
                        nc.tensor.matmul(
                            vps[j][0:48, :],
                            V[kt][:, 96 * j : 96 * j + 48],
                            attn[:, 2 * j * QB : (2 * j + 1) * QB],
                            start=(kt == 0),
                            stop=(kt == n_kt - 1),
                            tile_position=(0, 0),
                        )
                        nc.tensor.matmul(
                            vps[j][64:112, :],
                            V[kt][:, 96 * j + 48 : 96 * j + 96],
                            attn[:, (2 * j + 1) * QB : (2 * j + 2) * QB],
                            start=(kt == 0),
                            stop=(kt == n_kt - 1),
                            tile_position=(0, 64),
                            # disjoint partition range (64:112) of the same
                            # bank as the (0,0) group; group check is
                            # bank-granular and would false-positive
                            skip_group_check=True,
                        )
                if "pv" in ablate:
                    continue
                for j in range(4):
                    nc.vector.tensor_copy(values[j][0:48, qs], vps[j][0:48, :])
                    nc.vector.tensor_copy(
                        values[j][64:112, qs], vps[j][64:112, :]
                    )

        if "pv" in ablate:
            return
        # ---- output projection -------------------------------------------
        # Each head owns a full 512-f32-col PSUM region (start=True lazily
        # zeroes the whole region), 4 heads per [128, 2048] tile, two halves
        # per e-tile. Even heads read values partitions 0:48 (row group 0),
        # odd heads partitions 64:112 (row group 64, duplicated W_o^T rows).
        with tc.tile_pool(name="outps", bufs=2, space="PSUM") as op:
            for et in range(3):
                osb = outp.tile([128, S], f32, tag="osb", name="osb")
                for half in range(2):
                    ops_ = op.tile([128, 2048], f32, tag="out_ps", name="out_ps")
                    for hh in range(4):
                        h = 4 * half + hh
                        base = 0 if h % 2 == 0 else 64
                        rhs = values[h // 2][base : base + 48, :].rearrange(
                            "p (t i) -> p i t", i=8
                        )
                        for i in range(8):
                            nc.tensor.matmul(
                                ops_[:, 512 * hh : 512 * hh + T],
                                wo[
                                    base : base + 48,
                                    i * 384 + et * 128 : i * 384 + et * 128 + 128,
                                ],
                                rhs[:, i, :],
                                start=(i == 0),
                                stop=(i == 7),
                                tile_position=(base, 0),
                            )
                    ops_v = ops_[:, :].rearrange("p (hh q) -> p hh q", hh=4)[
                        :, :, 0:T
                    ]
                    osb_v = osb[
                        :, 4 * half * T : 4 * (half + 1) * T
                    ].rearrange("p (hh q) -> p hh q", hh=4)
                    nc.scalar.activation(
                        osb_v, ops_v, AF.Identity, bias=bo[et][:]
                    )
                nc.sync.dma_start(out_d[et * 128 : (et + 1) * 128, :], osb[:])


def build_body_v2(nc, tc, mybir, AF, persist, big2, small, outp,
                  xch, wqkch, wvch, wo, bo, out_d,
                  S, QB, n_qb, n_kt, n_st, SC, n_sc, T, scale,
                  f16, f32, gp_adds=1, interleave=True, filler=0, qkb=None, g_dve=False, xones=None):
    from contextlib import ExitStack as _ES
    _fes = _ES()

    def emit_filler(n):
        # dummy weight loads: pure PE-array activity (no PSUM write, own
        # SBUF read port) to keep the HAM activity monitor from
        # re-throttling the PE clock during ACT/DVE-paced stretches.
        for _ in range(n):
            nc.tensor.ldweights(wo[0:128, 0:512])
    """Rebalanced attention pipeline:
      - one D-tree add offloaded to GPSIMD (Pool) to unload DVE
      - values stored head-interleaved (i-major) so the out-projection
        streams a contiguous rhs instead of a stride-8 view
      - out-projection per-head rhs is then values[j][base:, i*T:(i+1)*T]
    """
    # ---- QKV projection + attention (interleaved) --------------------
    qkT = [persist.tile([128, S], f16, tag=f"qkT{t}", name=f"qkT{t}") for t in range(8)]
    V = [persist.tile([128, 384], f16, tag=f"V{st}", name=f"V{st}") for st in range(n_st)]
    values = [persist.tile([128, S], f16, tag=f"values{j}", name=f"values{j}") for j in range(4)]

    with tc.tile_pool(name="qkvps", bufs=2, space="PSUM") as qp:
        for sc in range(n_sc):
            for t in range(8):
                ps = qp.tile([128, SC], f32, tag="qk_ps", name="qk_ps", bufs=3)
                for c in range(3):
                    nc.tensor.matmul(
                        ps[:],
                        wqkch[c][:, t * 128 : (t + 1) * 128],
                        xch[c][:, sc * SC : (sc + 1) * SC],
                        start=(c == 0),
                        stop=(c == 2),
                    )
                # Q/K bias is per-partition in this layout: fold it into the
                # PSUM->SBUF copy instead of a 4th (1-row) matmul chunk.
                nc.vector.tensor_scalar_add(
                    qkT[t][:, sc * SC : (sc + 1) * SC], ps[:],
                    qkb[:, t : t + 1],
                )
            for st in range(sc * n_st // n_sc, (sc + 1) * n_st // n_sc):
                ps = qp.tile([128, 384], f32, tag="v_ps", name="v_ps")
                for c in range(4):
                    nc.tensor.matmul(
                        ps[:],
                        xones[:, st * 128 : (st + 1) * 128]
                        if c == 3
                        else xch[c][:, st * 128 : (st + 1) * 128],
                        wvch[c][:],
                        start=(c == 0),
                        stop=(c == 3),
                    )
                nc.vector.tensor_copy(V[st][:], ps[:])

    with (
        tc.tile_pool(name="scps", bufs=2, space="PSUM") as scp,
        tc.tile_pool(name="pvps", bufs=1, space="PSUM") as pvp,
    ):
        for qb in range(n_qb):
            qs = slice(qb * QB, (qb + 1) * QB)
            vps = [pvp.tile([128, QB], f32, tag=f"pv{j}", name=f"pv{j}") for j in range(4)]

            def emit_pv_j(attn_t, kt, j):
                nc.tensor.matmul(
                    vps[j][0:48, :],
                    V[kt][:, 96 * j : 96 * j + 48],
                    attn_t[:, 2 * j * QB : (2 * j + 1) * QB],
                    start=(kt == 0),
                    stop=(kt == n_kt - 1),
                    tile_position=(0, 0),
                )
                nc.tensor.matmul(
                    vps[j][64:112, :],
                    V[kt][:, 96 * j + 48 : 96 * j + 96],
                    attn_t[:, (2 * j + 1) * QB : (2 * j + 2) * QB],
                    start=(kt == 0),
                    stop=(kt == n_kt - 1),
                    tile_position=(0, 64),
                    skip_group_check=True,
                )

            def emit_pv(attn_t, kt):
                for j in range(4):
                    emit_pv_j(attn_t, kt, j)

            # Software pipeline, per iteration kt:
            #   PE:  scores(kt), then PV(kt-1) -- PV issued after the next
            #        tile's score matmuls so the PE's in-order queue isn't
            #        head-of-line blocked on kt-1's normalized weights
            #   ACT: exps(kt), then Ln/ExpG(kt-1)
            #   DVE: D-tree adds(kt), then in-place normalize muls(kt-1)
            # (Deeper deferral measures WORSE: longer PE idle stretches
            # trigger more HAM clock re-throttles.)
            pending = None
            for kt in range(n_kt):
                if True:
                    ks = slice(kt * 128, (kt + 1) * 128)
                    exp_sb = big2.tile([128, 8 * QB], f16, tag="exp", name="exp_sb")
                    for j in range(4):
                        # each half sits in its own PSUM bank (512 f32 cols)
                        sps = scp.tile([128, 1024], f32, tag="sc_ps", name="sc_ps")
                        nc.tensor.matmul(
                            sps[:, 0:QB],
                            qkT[2 * j][0:48, ks],
                            qkT[2 * j + 1][0:48, qs],
                            tile_position=(0, 0),
                        )
                        nc.tensor.matmul(
                            sps[:, 512 : 512 + QB],
                            qkT[2 * j][64:112, ks],
                            qkT[2 * j + 1][64:112, qs],
                            tile_position=(64, 0),
                        )
                        sps_v = sps[:, :].rearrange("p (b q) -> p b q", b=2)[
                            :, :, 0:QB
                        ]
                        exp_v = exp_sb[
                            :, 2 * j * QB : (2 * j + 2) * QB
                        ].rearrange("p (h q) -> p h q", h=2)
                        nc.scalar.activation(exp_v, sps_v, AF.Exp, scale=scale)
                        # interleave the previous tile's PV pairs into the
                        # score loop: the sps-ring WAR makes scores j>=2 wait
                        # on this tile's early exps, and the PV work fills
                        # that PE stall window
                        if pending is not None:
                            if j == 1:
                                emit_pv_j(pending[0], pending[1], 0)
                                emit_pv_j(pending[0], pending[1], 1)
                            elif j == 2:
                                emit_pv_j(pending[0], pending[1], 2)
                            elif j == 3:
                                emit_pv_j(pending[0], pending[1], 3)
                # D = sum over heads: 4-op tree
                D2 = small.tile([128, 4 * QB], f16, tag="D2", name="D2")
                nc.vector.tensor_add(
                    D2[:, 0 : 2 * QB],
                    exp_sb[:, 0 : 2 * QB],
                    exp_sb[:, 2 * QB : 4 * QB],
                )
                nc.vector.tensor_add(
                    D2[:, 2 * QB : 4 * QB],
                    exp_sb[:, 4 * QB : 6 * QB],
                    exp_sb[:, 6 * QB : 8 * QB],
                )
                nc.vector.tensor_add(
                    D2[:, 0 : 2 * QB], D2[:, 0 : 2 * QB], D2[:, 2 * QB : 4 * QB]
                )
                Dt = small.tile([128, QB], f16, tag="D", name="Dt")
                nc.vector.tensor_add(Dt[:], D2[:, 0:QB], D2[:, QB : 2 * QB])
                Gt = small.tile([128, QB], f16, tag="G", name="Gt")
                lnD = small.tile([128, QB], f16, tag="lnD", name="lnD")
                nc.scalar.activation(lnD[:], Dt[:], AF.Ln)
                nc.scalar.activation(Gt[:], lnD[:], AF.Exp, scale=-1.0)
                g_b = Gt[:].rearrange("p (o q) -> p o q", o=1).broadcast_to(
                    [128, 4, QB]
                )
                for w in range(2):
                    # in place: DVE is in-order, the D-tree adds above
                    # already read these columns
                    nc.vector.tensor_mul(
                        exp_sb[:, 4 * w * QB : 4 * (w + 1) * QB].rearrange(
                            "p (h q) -> p h q", h=4
                        ),
                        exp_sb[:, 4 * w * QB : 4 * (w + 1) * QB].rearrange(
                            "p (h q) -> p h q", h=4
                        ),
                        g_b,
                    )
                pending = (exp_sb, kt)
            emit_pv(*pending)
            # unload PV accumulators. With interleave=True, scatter the q
            # columns i-major (col i*T + t holds q=8t+i) so the out
            # projection's per-i rhs blocks are contiguous.
            U = QB // 8
            for j in range(4):
                for base in (0, 64):
                    src = vps[j][base : base + 48, :]
                    if interleave:
                        src_v = src.rearrange("p (u i) -> p i u", i=8)
                        dst_v = (
                            values[j][base : base + 48, :]
                            .rearrange("p (i t) -> p i t", i=8)[
                                :, :, qb * U : (qb + 1) * U
                            ]
                        )
                        nc.vector.tensor_copy(dst_v, src_v)
                    else:
                        nc.vector.tensor_copy(
                            values[j][base : base + 48, qs], src
                        )

    # ---- output projection -------------------------------------------
    with tc.tile_pool(name="outps", bufs=2, space="PSUM") as op:
        for et in range(3):
            osb = outp.tile([128, S], f32, tag="osb", name="osb")
            for half in range(2):
                # each head owns a full 512-f32-col PSUM region: accumulation
                # group zeroing is bank-granular
                ops_ = op.tile([128, 2048], f32, tag="out_ps", name="out_ps")
                for hh in range(4):
                    h = 4 * half + hh
                    base = 0 if h % 2 == 0 else 64
                    vt = values[h // 2]
                    for i in range(8):
                        if interleave:
                            rhs = vt[base : base + 48, i * T : (i + 1) * T]
                        else:
                            rhs = vt[base : base + 48, :].rearrange(
                                "p (t i) -> p i t", i=8
                            )[:, i, :]
                        nc.tensor.matmul(
                            ops_[:, 512 * hh : 512 * hh + T],
                            wo[
                                base : base + 48,
                                i * 384 + et * 128 : i * 384 + et * 128 + 128,
                            ],
                            rhs,
                            start=(i == 0),
                            stop=(i == 7),
                            tile_position=(base, 0),
                        )
                ops_v = ops_[:, :].rearrange("p (hh q) -> p hh q", hh=4)[
                    :, :, 0:T
                ]
                osb_v = osb[
                    :, 4 * half * T : 4 * (half + 1) * T
                ].rearrange("p (hh q) -> p hh q", hh=4)
                nc.scalar.activation(
                    osb_v, ops_v, AF.Identity, bias=bo[et][:]
                )
            nc.sync.dma_start(out_d[et * 128 : (et + 1) * 128, :], osb[:])


def _get_program(S=S_FULL):
    key = ("nc", S)
    if key not in _CACHE:
        _CACHE[key] = build_program(S)
    return _CACHE[key]


def kernel(x, W_qkv, b_qkv, W_o, b_o):
    from concourse import bass_utils

    x = np.asarray(x, dtype=np.float32)
    W_qkv = np.asarray(W_qkv, dtype=np.float32)
    b_qkv = np.asarray(b_qkv, dtype=np.float32)
    W_o = np.asarray(W_o, dtype=np.float32)
    b_o = np.asarray(b_o, dtype=np.float32)
    Bx, S, _ = x.shape

    wqkT, wvT, woT, bo, qkb = _pack_host(W_qkv, b_qkv, W_o, b_o)
    in_maps = []
    for b in range(Bx):
        xT = np.ones((385, S), np.float32)
        xT[:384] = x[b].T
        in_maps.append(
            {
                "xT": xT.astype(np.float16),
                "wqkT": wqkT,
                "wvT": wvT,
                "woT": woT,
                "bo": bo,
                "qkb": qkb,
            }
        )

    nc = _get_program(S)
    res = bass_utils.run_bass_kernel_spmd(nc, in_maps, core_ids=list(range(Bx)))
    out = np.stack([np.ascontiguousarray(r["outT"].T) for r in res.results])
    return out.astype(np.float32)

